# revision 55
# baseline (speedup 1.0000x reference)
"""Trainium2 Bass kernel for nn_Attention_9844065042780.

Sharding: expert-parallel over the K=8 independent groups, one group per
NeuronCore (8 cores).  Each core receives the full activations x (reordered
host-side), the full conv_w (to compute the shared softmax attention maps
and the shared orthogonality loss), and only its own group's
dimred/Wo/Wg weights.  Outputs are disjoint per-core slices (hyp[:,k,:],
conf[:,k]) plus the (identical on every core) loss, gathered host-side.

Per-core math (k = this core's group):
  z[k', hw, b]   = sum_n conv_w[k', n] x[b, n, hw]            (PE)
  ah[k', hw, b]  = softmax_k'(z)                              (ACT exp, PE sum, DVE recip/mul)
  y[(hw,n), b]   = ah[k, hw, b] * x[b, n, hw]                 (PE row-broadcast + DVE mul)
  dim_red[b, m]  = sum_{hw,n} y * wd[m, n, hw] + db[m]        (PE, 417 accumulating matmuls)
  hyp[c, b]      = sum_n Wo[c, n] dim_red[b, n] + Wo_b[c]     (PE + ACT bias)
  conf[b]        = tanh(sum_n Wg[n] dim_red[b, n] + Wg_b)     (PE + ACT)
  loss[b]        = ||A_b^T A_b||_F^2 - sum((H^T A_b)^2),  A_b = ah[:, :, :, b]  (PE/ACT/DVE)

Layouts (host-prepared, hw padded 196->208 so slices are uniform 16*32=512):
  x_t  [2, 208, 128, 32]  bf16   x_t[nh, hw, nl, b] = x[b, nh*128+nl, hw]
  wd_t [2, 208, 128, 256] bf16   wd_t[nh, hw, nl, m] = dimred_w[k, m, nh*128+nl, hw]
"""

import os
import numpy as np
import ml_dtypes
from contextlib import ExitStack

from concourse import bass, bacc, tile, mybir
from concourse.bass_utils import run_bass_kernel_spmd

F32 = mybir.dt.float32
BF16 = mybir.dt.bfloat16
AF = mybir.ActivationFunctionType
ALU = mybir.AluOpType
AX = mybir.AxisListType

B, N, H, W, K, C = 32, 256, 14, 14, 8, 1000
HW, HWP, CP = 196, 196, 1024
SW = 14                      # hw positions per slice (one h row)
NSL = HWP // SW              # 14 slices
SLW = SW * B                 # 448 free columns per slice
NCORES = 8


def build_graph():
    nc = bacc.Bacc("TRN2", target_bir_lowering=False, debug=False)

    def inp(name, shape, dtype):
        return nc.dram_tensor(name, shape, dtype, kind="ExternalInput").ap()

    def outp(name, shape, dtype):
        return nc.dram_tensor(name, shape, dtype, kind="ExternalOutput").ap()

    x_d = inp("x_t", [2, 128, HWP, B], BF16)
    wd_d = inp("wd_t", [128, 2, HWP, N], BF16)
    cw_d = inp("cw_t", [2, 128, K], BF16)
    wo_d = inp("wo_t", [2, 128, CP], F32)
    wob_d = inp("wob_t", [CP // 128, 128], F32)
    wg_d = inp("wg_t", [2, 128, 1], F32)
    wgb_d = inp("wgb_t", [1, 1], F32)
    db_d = inp("db_t", [1, N], F32)
    ones8_d = inp("ones8", [8, 8], BF16)
    selk_d = inp("selk", [8, 128], BF16)
    hsel_d = inp("hsel", [K * H, K], F32)
    ones14_d = inp("ones14", [14, 1], F32)
    mones8_d = inp("mones8", [8, 1], F32)
    onesb_d = inp("onesb", [1, B], F32)
    id32_d = inp("id32", [32, 32], F32)

    hyp_d = outp("out_hyp", [CP // 128, 128, B], F32)
    conf_d = outp("out_conf", [1, B], F32)
    loss_d = outp("out_loss", [1, B], F32)

    with tile.TileContext(nc) as tc, ExitStack() as ctx:
        const = ctx.enter_context(tc.tile_pool(name="const", bufs=1))
        persist = ctx.enter_context(tc.tile_pool(name="persist", bufs=1))
        dram = ctx.enter_context(tc.tile_pool(name="dram", bufs=1, space="DRAM"))
        dr_pool = ctx.enter_context(
            tc.tile_pool(name="dr_psum", bufs=1, space="PSUM")
        )

        # PE warm-up first in program order: memset is gpsimd's first
        # instruction and the matmuls open the HAM clock gate
        # (1.2 -> 2.4 GHz) while the DMA pipeline ramps.
        with ExitStack() as wctx:
            wup = wctx.enter_context(tc.tile_pool(name="wup", bufs=1))
            wupp = wctx.enter_context(
                tc.tile_pool(name="wupp", bufs=1, space="PSUM"))
            wu_sb = wup.tile([128, 512], BF16)
            nc.gpsimd.memset(wu_sb, 0.0)
            wu_ps = wupp.tile([128, 512], F32, space="PSUM")
            for _ in range(12):
                nc.tensor.matmul(wu_ps, lhsT=wu_sb[:, 0:128], rhs=wu_sb,
                                 start=True, stop=True)

        # ---- constants into SBUF ----
        # Main-loop-critical consts go FIRST on the sync HWDGE ring (tiny);
        # tail-only consts go on the gpsimd SWDGE ring so neither the sync
        # ring (wd stream) nor the scalar engine (exp/copies) is blocked.
        cw_sb = const.tile([128, 2 * K], BF16)
        nc.sync.dma_start(cw_sb.rearrange("p (t c) -> p t c", t=2),
                          cw_d.rearrange("t p c -> p t c"))
        db_sb = const.tile([1, N], F32)
        nc.sync.dma_start(db_sb, db_d)
        ones8_sb = const.tile([8, 8], BF16)
        nc.sync.dma_start(ones8_sb, ones8_d)
        selk_sb = const.tile([8, 128], BF16)
        nc.sync.dma_start(selk_sb, selk_d)
        onesb_sb = const.tile([1, B], F32)
        nc.sync.dma_start(onesb_sb, onesb_d)

        wo_sb = const.tile([128, 2 * CP], F32)
        nc.gpsimd.dma_start(wo_sb.rearrange("p (t c) -> p t c", t=2),
                            wo_d.rearrange("t p c -> p t c"))
        wob_sb = const.tile([128, CP // 128], F32)
        nc.gpsimd.dma_start(wob_sb, wob_d.rearrange("c p -> p c"))
        wg_sb = const.tile([128, 2], F32)
        nc.gpsimd.dma_start(wg_sb.rearrange("p (t one) -> p t one", t=2),
                            wg_d.rearrange("t p one -> p t one"))
        wgb_sb = const.tile([1, 1], F32)
        nc.gpsimd.dma_start(wgb_sb, wgb_d)
        hsel_sb = const.tile([K * H, K], F32)
        nc.gpsimd.dma_start(hsel_sb, hsel_d)
        ones14_sb = const.tile([14, 1], F32)
        nc.gpsimd.dma_start(ones14_sb, ones14_d)
        mones8_sb = const.tile([8, 1], F32)
        nc.gpsimd.dma_start(mones8_sb, mones8_d)
        id32_sb = const.tile([32, 32], F32)
        nc.gpsimd.dma_start(id32_sb, id32_d)

        # full attention map, all K groups: [8, (hw, b)] fp32
        ah_full = persist.tile([K, HWP * B], F32)

        # x resident in SBUF: [128, (u, hw, b)].  Loaded on the sync ring in
        # quarter-DMAs interleaved with the first wd slices (emitted in
        # stage_z below), first quarter up-front so slice 0 unblocks early.
        x_all = persist.tile([128, 2 * HWP * B], BF16)
        XH = HWP * B
        QHW = 49  # hw positions per x quarter-load (49*4 = 196)

        def load_x_quarter(q):
            lo, hi = q * QHW, (q + 1) * QHW
            for u in range(2):
                nc.sync.dma_start(
                    x_all[:, u * XH + lo * B:u * XH + hi * B],
                    x_d[u, :, lo:hi, :])

        load_x_quarter(0)

        # dim_red accumulator [b, m]
        dimred_ps = dr_pool.tile([B, N], F32, space="PSUM")
        # bias seeding matmul: dimred[b, m] = 1 * db[m]
        nc.tensor.matmul(
            dimred_ps, lhsT=onesb_sb, rhs=db_sb, start=True, stop=False,
            skip_group_check=True,
        )

        with ExitStack() as mctx:
            wdp = mctx.enter_context(tc.tile_pool(name="wdp", bufs=6))
            ep = mctx.enter_context(tc.tile_pool(name="ep", bufs=4))
            abf = mctx.enter_context(tc.tile_pool(name="abf", bufs=3))
            rp = mctx.enter_context(tc.tile_pool(name="rp", bufs=3))
            yp = mctx.enter_context(tc.tile_pool(name="yp", bufs=4))
            zp = mctx.enter_context(tc.tile_pool(name="zp", bufs=3, space="PSUM"))
            sp = mctx.enter_context(tc.tile_pool(name="sp", bufs=2, space="PSUM"))
            abp = mctx.enter_context(tc.tile_pool(name="abp", bufs=2, space="PSUM"))

            def xw(s, u):
                return x_all[:, u * HWP * B + s * SW * B:
                             u * HWP * B + (s + 1) * SW * B]

            # Software-pipelined emission: PE sees z MMs of slice s+2 and
            # ssum of s+1 between dependent ops of slice s, so it never
            # stalls in-order on the cross-engine softmax chain.
            state = {}

            def stage_z(s):
                # one 1.8MB wd DMA per slice, prefetched 2 slices ahead;
                # remaining x quarters ride between the first wd slices
                wd_sb = wdp.tile([128, 2 * SW * N], BF16, tag="wd",
                                 name=f"wd_{s}")
                nc.sync.dma_start(
                    wd_sb.rearrange("p (u t m) -> p u (t m)", u=2, t=SW),
                    wd_d[:, :, s * SW:(s + 1) * SW, :])
                if 1 <= s <= 3:
                    load_x_quarter(s)
                z_ps = zp.tile([K, SLW], F32, tag="z", space="PSUM",
                               name=f"z_{s}")
                nc.tensor.matmul(z_ps, lhsT=cw_sb[:, 0:K], rhs=xw(s, 0),
                                 start=True, stop=False)
                nc.tensor.matmul(z_ps, lhsT=cw_sb[:, K:2 * K], rhs=xw(s, 1),
                                 start=False, stop=True)
                e_sb = ep.tile([K, SLW], F32, tag="e", name=f"e_{s}")
                nc.scalar.activation(e_sb, z_ps, AF.Exp)
                # bf16 exp copy: lets the softmax-sum matmul run at 1 cyc/row
                eb_sb = ep.tile([K, SLW], BF16, tag="eb", name=f"eb_{s}")
                nc.scalar.activation(eb_sb, z_ps, AF.Exp)
                state[s] = (wd_sb, e_sb, eb_sb)

            def stage_mid(s):
                wd_sb, e_sb, eb_sb = state[s]
                ssum_ps = sp.tile([K, SLW], F32, tag="ss", space="PSUM",
                                  name=f"ss_{s}")
                nc.tensor.matmul(ssum_ps, lhsT=ones8_sb, rhs=eb_sb,
                                 start=True, stop=True)
                r_sb = rp.tile([K, SLW], F32, tag="r", name=f"r_{s}")
                nc.vector.reciprocal_approx_fast(r_sb, ssum_ps)
                ahw = ah_full[:, s * SLW:(s + 1) * SLW]
                nc.vector.tensor_tensor(out=ahw, in0=e_sb, in1=r_sb,
                                        op=ALU.mult)
                # bf16 copy (on ACT) so the broadcast matmul is 1 cyc/row
                ah_bf = abf.tile([K, SLW], BF16, tag="ahbf", name=f"abf_{s}")
                nc.scalar.copy(ah_bf, ahw)
                state[s] = (wd_sb, ah_bf)

            def stage_heavy(s):
                wd_sb, ah_bf = state.pop(s)
                ahb_ps = abp.tile([128, SLW], F32, tag="ab", space="PSUM",
                                  name=f"ab_{s}")
                nc.tensor.matmul(ahb_ps, lhsT=selk_sb, rhs=ah_bf,
                                 start=True, stop=True)
                last_slice = s == NSL - 1
                for nh in range(2):
                    y_sb = yp.tile([128, SLW], BF16, tag="y",
                                   name=f"y_{s}_{nh}")
                    nc.vector.tensor_tensor(out=y_sb, in0=xw(s, nh),
                                            in1=ahb_ps, op=ALU.mult)
                    for j in range(SW):
                        nc.tensor.matmul(
                            dimred_ps,
                            lhsT=y_sb[:, j * B:(j + 1) * B],
                            rhs=wd_sb[:, (nh * SW + j) * N:
                                      (nh * SW + j + 1) * N],
                            start=False,
                            stop=(last_slice and nh == 1 and j == SW - 1),
                            skip_group_check=True,
                        )

            LEAD = 2
            for s in range(NSL + LEAD):
                if s < NSL:
                    stage_z(s)
                if 1 <= s and s - 1 < NSL:
                    stage_mid(s - 1)
                if s >= LEAD:
                    stage_heavy(s - LEAD)

        # ---------------- tail ----------------
        with ExitStack() as tctx:
            tp = tctx.enter_context(tc.tile_pool(name="tail_sb", bufs=1))
            tpp = tctx.enter_context(
                tc.tile_pool(name="tail_ps", bufs=1, space="PSUM")
            )
            hp = tctx.enter_context(
                tc.tile_pool(name="hyp_ps", bufs=2, space="PSUM")
            )
            hs = tctx.enter_context(tc.tile_pool(name="hyp_sb", bufs=2))

            # dim_red -> SBUF f32, then transpose to [n, b]
            dr_sb = tp.tile([B, N], F32)
            nc.vector.tensor_copy(dr_sb, dimred_ps)
            drT_sb = tp.tile([128, 2 * B], F32)
            for nh in range(2):
                drT_ps = tpp.tile([128, B], F32, tag="drT", space="PSUM")
                nc.tensor.transpose(
                    drT_ps, dr_sb[:, nh * 128:(nh + 1) * 128], id32_sb
                )
                nc.vector.tensor_copy(drT_sb[:, nh * B:(nh + 1) * B], drT_ps)

            # hyp
            for c in range(CP // 128):
                hyp_ps = hp.tile([128, B], F32, tag="hyp", space="PSUM")
                nc.tensor.matmul(
                    hyp_ps, lhsT=wo_sb[:, c * 128:(c + 1) * 128],
                    rhs=drT_sb[:, 0:B], start=True, stop=False,
                )
                nc.tensor.matmul(
                    hyp_ps, lhsT=wo_sb[:, CP + c * 128:CP + (c + 1) * 128],
                    rhs=drT_sb[:, B:2 * B], start=False, stop=True,
                )
                hyp_sb = hs.tile([128, B], F32, tag="hyps")
                nc.scalar.activation(hyp_sb, hyp_ps, AF.Identity,
                                     bias=wob_sb[:, c:c + 1])
                nc.sync.dma_start(hyp_d[c], hyp_sb)

            # conf
            conf_ps = tpp.tile([1, B], F32, tag="conf", space="PSUM")
            nc.tensor.matmul(conf_ps, lhsT=wg_sb[:, 0:1], rhs=drT_sb[:, 0:B],
                             start=True, stop=False)
            nc.tensor.matmul(conf_ps, lhsT=wg_sb[:, 1:2], rhs=drT_sb[:, B:2 * B],
                             start=False, stop=True)
            conf_sb = tp.tile([1, B], F32)
            nc.scalar.activation(conf_sb, conf_ps, AF.Tanh, bias=wgb_sb[:, 0:1])
            nc.sync.dma_start(conf_d, conf_sb)

            # ---- loss ----
            # regather ah (hw < 196 region) to [(k h), (w b)] via DRAM bounce
            # (gpsimd ring: runs as soon as ah_full completes, not behind wd)
            ah_dram = dram.tile([K, H * W * B], F32, space="DRAM")
            nc.gpsimd.dma_start(ah_dram, ah_full[:, 0:H * W * B])
            A2 = tp.tile([K * H, W * B], F32)
            nc.gpsimd.dma_start(
                A2, ah_dram.rearrange("k (h rest) -> (k h) rest", h=H)
            )
            A2v = A2.rearrange("p (w b) -> p w b", b=B)
            G_ps = tpp.tile([W, B * W], F32, tag="G", space="PSUM")
            for b in range(B):
                ab = A2v[:, :, b:b + 1]
                nc.tensor.matmul(G_ps[:, b * W:(b + 1) * W], lhsT=ab, rhs=ab,
                                 start=True, stop=True, skip_group_check=True)
            S_ps = tpp.tile([K, W * B], F32, tag="S", space="PSUM")
            nc.tensor.matmul(S_ps, lhsT=hsel_sb, rhs=A2, start=True, stop=True)
            Gsq = tp.tile([W, B * W], F32)
            nc.scalar.activation(Gsq, G_ps, AF.Square)
            Ssq = tp.tile([K, W * B], F32)
            nc.scalar.activation(Ssq, S_ps, AF.Square)
            Gred = tp.tile([W, B], F32)
            nc.vector.tensor_reduce(
                Gred, Gsq.rearrange("p (b v) -> p b v", b=B),
                axis=AX.X, op=ALU.add,
            )
            Sred = tp.tile([K, B], F32)
            nc.vector.tensor_reduce(
                Sred, Ssq.rearrange("p (w b) -> p b w", b=B),
                axis=AX.X, op=ALU.add,
            )
            l_ps = tpp.tile([1, B], F32, tag="l", space="PSUM")
            nc.tensor.matmul(l_ps, lhsT=ones14_sb, rhs=Gred,
                             start=True, stop=False)
            nc.tensor.matmul(l_ps, lhsT=mones8_sb, rhs=Sred,
                             start=False, stop=True)
            loss_sb = tp.tile([1, B], F32)
            nc.vector.tensor_copy(loss_sb, l_ps)
            nc.sync.dma_start(loss_d, loss_sb)

    return nc


def _bf16(a):
    return np.ascontiguousarray(a.astype(ml_dtypes.bfloat16))


def build_host_inputs(x, conv_w, dimred_w, dimred_b, Wo_w, Wo_b, Wg_w, Wg_b):
    """Returns in_maps: one dict per core."""
    x = np.asarray(x, np.float32)
    conv_w = np.asarray(conv_w, np.float32)
    dimred_w = np.asarray(dimred_w, np.float32)
    dimred_b = np.asarray(dimred_b, np.float32)
    Wo_w = np.asarray(Wo_w, np.float32)
    Wo_b = np.asarray(Wo_b, np.float32)
    Wg_w = np.asarray(Wg_w, np.float32)
    Wg_b = np.asarray(Wg_b, np.float32)

    # x_t[nh, nl, hw, b] = x[b, nh*128+nl, hw]  (partition-major contiguous)
    xt = x.transpose(1, 2, 3, 0).reshape(N, HW, B)          # [n, hw, b]
    xt = _bf16(xt.reshape(2, 128, HWP, B))

    # conv_w^T [2, 128, K]
    cwt = _bf16(conv_w.T.reshape(2, 128, K))

    shared = {
        "x_t": xt,
        "cw_t": cwt,
        "ones8": _bf16(np.ones((8, 8), np.float32)),
        "hsel": np.repeat(np.eye(K, dtype=np.float32), H, axis=0),
        "ones14": np.ones((14, 1), np.float32),
        "mones8": -np.ones((8, 1), np.float32),
        "onesb": np.ones((1, B), np.float32),
        "id32": np.eye(32, dtype=np.float32),
    }

    in_maps = []
    for k in range(NCORES):
        # wd_t[nl, nh, hw, m] = dimred_w[k, m, n, hw]  (partition-outermost)
        wd = dimred_w[k].reshape(N, N, HW).transpose(1, 2, 0)   # [n, hw, m]
        wd = wd.reshape(2, 128, HWP, N).transpose(1, 0, 2, 3)   # [128, 2, hw, m]
        wo = np.zeros((CP, N), np.float32)
        wo[:C] = Wo_w[k]
        wob = np.zeros((CP,), np.float32)
        wob[:C] = Wo_b[k]
        selk = np.zeros((8, 128), np.float32)
        selk[k] = 1.0
        m = dict(shared)
        m.update({
            "wd_t": _bf16(wd),
            "wo_t": np.ascontiguousarray(wo.T.reshape(2, 128, CP)),
            "wob_t": wob.reshape(CP // 128, 128),
            "wg_t": np.ascontiguousarray(Wg_w[k].reshape(2, 128, 1)),
            "wgb_t": np.full((1, 1), Wg_b[k], np.float32),
            "db_t": dimred_b[k].reshape(1, N),
            "selk": _bf16(selk),
        })
        in_maps.append(m)
    return in_maps


def assemble_outputs(results):
    hyp = np.stack(
        [r["out_hyp"].reshape(CP, B)[:C].T for r in results], axis=1
    )                                                   # [B, K, C]
    conf = np.stack([r["out_conf"][0] for r in results], axis=1)[..., None]
    loss = results[0]["out_loss"][0][:, None]           # [B, 1]
    return (
        np.ascontiguousarray(hyp, np.float32),
        np.ascontiguousarray(conf, np.float32),
        np.ascontiguousarray(loss, np.float32),
    )


_GRAPH_CACHE = {}


def get_graph():
    if "nc" not in _GRAPH_CACHE:
        nc = build_graph()
        nc.finalize()
        _GRAPH_CACHE["nc"] = nc
    return _GRAPH_CACHE["nc"]


def kernel(**inputs):
    nc = get_graph()
    in_maps = build_host_inputs(**inputs)
    res = run_bass_kernel_spmd(nc, in_maps, core_ids=list(range(NCORES)))
    return assemble_outputs(res.results)


# revision 64
# speedup vs baseline: 1.0564x; 1.0564x over previous
"""Trainium2 Bass kernel for nn_Attention_9844065042780.

Sharding: expert-parallel over the K=8 independent groups, one group per
NeuronCore (8 cores).  Each core receives the full activations x (reordered
host-side), the full conv_w (to compute the shared softmax attention maps
and the shared orthogonality loss), and only its own group's
dimred/Wo/Wg weights.  Outputs are disjoint per-core slices (hyp[:,k,:],
conf[:,k]) plus the (identical on every core) loss, gathered host-side.

Per-core math (k = this core's group):
  z[k', hw, b]   = sum_n conv_w[k', n] x[b, n, hw]            (PE)
  ah[k', hw, b]  = softmax_k'(z)                              (ACT exp, PE sum, DVE recip/mul)
  y[(hw,n), b]   = ah[k, hw, b] * x[b, n, hw]                 (PE row-broadcast + DVE mul)
  dim_red[b, m]  = sum_{hw,n} y * wd[m, n, hw] + db[m]        (PE, 417 accumulating matmuls)
  hyp[c, b]      = sum_n Wo[c, n] dim_red[b, n] + Wo_b[c]     (PE + ACT bias)
  conf[b]        = tanh(sum_n Wg[n] dim_red[b, n] + Wg_b)     (PE + ACT)
  loss[b]        = ||A_b^T A_b||_F^2 - sum((H^T A_b)^2),  A_b = ah[:, :, :, b]  (PE/ACT/DVE)

Layouts (host-prepared, hw padded 196->208 so slices are uniform 16*32=512):
  x_t  [2, 208, 128, 32]  bf16   x_t[nh, hw, nl, b] = x[b, nh*128+nl, hw]
  wd_t [2, 208, 128, 256] bf16   wd_t[nh, hw, nl, m] = dimred_w[k, m, nh*128+nl, hw]
"""

import os
import numpy as np
import ml_dtypes
from contextlib import ExitStack

from concourse import bass, bacc, tile, mybir
from concourse.bass_utils import run_bass_kernel_spmd

F32 = mybir.dt.float32
BF16 = mybir.dt.bfloat16
AF = mybir.ActivationFunctionType
ALU = mybir.AluOpType
AX = mybir.AxisListType

B, N, H, W, K, C = 32, 256, 14, 14, 8, 1000
HW, HWP, CP = 196, 196, 1024
SW = 14                      # hw positions per slice (one h row)
NSL = HWP // SW              # 14 slices
SLW = SW * B                 # 448 free columns per slice
NCORES = 8


def build_graph():
    nc = bacc.Bacc("TRN2", target_bir_lowering=False, debug=False)

    def inp(name, shape, dtype):
        return nc.dram_tensor(name, shape, dtype, kind="ExternalInput").ap()

    def outp(name, shape, dtype):
        return nc.dram_tensor(name, shape, dtype, kind="ExternalOutput").ap()

    x_d = inp("x_t", [2, 128, HWP, B], BF16)
    wd_d = inp("wd_t", [128, 2, HWP, N], BF16)
    cw_d = inp("cw_t", [2, 128, K], BF16)
    wo_d = inp("wo_t", [2, 128, CP], F32)
    wob_d = inp("wob_t", [CP // 128, 128], F32)
    wg_d = inp("wg_t", [2, 128, 1], F32)
    wgb_d = inp("wgb_t", [1, 1], F32)
    db_d = inp("db_t", [1, N], F32)
    ones8_d = inp("ones8", [8, 8], BF16)
    selk_d = inp("selk", [8, 128], BF16)
    hsel_d = inp("hsel", [K * H, K], F32)
    ones14_d = inp("ones14", [14, 1], F32)
    mones8_d = inp("mones8", [8, 1], F32)
    onesb_d = inp("onesb", [1, B], F32)
    id32_d = inp("id32", [32, 32], F32)
    sel4_d = inp("sel4", [128, B], F32)

    hyp_d = outp("out_hyp", [CP // 128, 128, B], F32)
    conf_d = outp("out_conf", [1, B], F32)
    loss_d = outp("out_loss", [1, B], F32)

    with tile.TileContext(nc) as tc, ExitStack() as ctx:
        const = ctx.enter_context(tc.tile_pool(name="const", bufs=1))
        persist = ctx.enter_context(tc.tile_pool(name="persist", bufs=1))
        dram = ctx.enter_context(tc.tile_pool(name="dram", bufs=1, space="DRAM"))
        dr_pool = ctx.enter_context(
            tc.tile_pool(name="dr_psum", bufs=1, space="PSUM")
        )

        # PE warm-up first in program order: memset is gpsimd's first
        # instruction and the matmuls open the HAM clock gate
        # (1.2 -> 2.4 GHz) while the DMA pipeline ramps.
        with ExitStack() as wctx:
            wup = wctx.enter_context(tc.tile_pool(name="wup", bufs=1))
            wupp = wctx.enter_context(
                tc.tile_pool(name="wupp", bufs=1, space="PSUM"))
            wu_sb = wup.tile([128, 512], BF16)
            nc.gpsimd.memset(wu_sb, 0.0)
            wu_ps = wupp.tile([128, 512], F32, space="PSUM")
            for _ in range(12):
                nc.tensor.matmul(wu_ps, lhsT=wu_sb[:, 0:128], rhs=wu_sb,
                                 start=True, stop=True)

        # ---- constants into SBUF ----
        # Main-loop-critical consts go FIRST on the sync HWDGE ring (tiny);
        # tail-only consts go on the gpsimd SWDGE ring so neither the sync
        # ring (wd stream) nor the scalar engine (exp/copies) is blocked.
        cw_sb = const.tile([128, 2 * K], BF16)
        nc.sync.dma_start(cw_sb.rearrange("p (t c) -> p t c", t=2),
                          cw_d.rearrange("t p c -> p t c"))
        db_sb = const.tile([1, N], F32)
        nc.sync.dma_start(db_sb, db_d)
        ones8_sb = const.tile([8, 8], BF16)
        nc.sync.dma_start(ones8_sb, ones8_d)
        selk_sb = const.tile([8, 128], BF16)
        nc.sync.dma_start(selk_sb, selk_d)
        onesb_sb = const.tile([1, B], F32)
        nc.sync.dma_start(onesb_sb, onesb_d)

        wo_sb = const.tile([128, 2 * CP], F32)
        nc.gpsimd.dma_start(wo_sb.rearrange("p (t c) -> p t c", t=2),
                            wo_d.rearrange("t p c -> p t c"))
        wob_sb = const.tile([128, CP // 128], F32)
        nc.gpsimd.dma_start(wob_sb, wob_d.rearrange("c p -> p c"))
        wg_sb = const.tile([128, 2], F32)
        nc.gpsimd.dma_start(wg_sb.rearrange("p (t one) -> p t one", t=2),
                            wg_d.rearrange("t p one -> p t one"))
        wgb_sb = const.tile([1, 1], F32)
        nc.gpsimd.dma_start(wgb_sb, wgb_d)
        hsel_sb = const.tile([K * H, K], F32)
        nc.gpsimd.dma_start(hsel_sb, hsel_d)
        ones14_sb = const.tile([14, 1], F32)
        nc.gpsimd.dma_start(ones14_sb, ones14_d)
        mones8_sb = const.tile([8, 1], F32)
        nc.gpsimd.dma_start(mones8_sb, mones8_d)
        id32_sb = const.tile([32, 32], F32)
        nc.gpsimd.dma_start(id32_sb, id32_d)
        sel4_sb = const.tile([128, B], F32)
        nc.gpsimd.dma_start(sel4_sb, sel4_d)

        # full attention map, all K groups: [8, (hw, b)] fp32
        ah_full = persist.tile([K, HWP * B], F32)

        # x resident in SBUF: [128, (u, hw, b)].  Loaded on the sync ring in
        # quarter-DMAs interleaved with the first wd slices (emitted in
        # stage_z below), first quarter up-front so slice 0 unblocks early.
        x_all = persist.tile([128, 2 * HWP * B], BF16)
        XH = HWP * B
        QHW = 49  # hw positions per x quarter-load (49*4 = 196)

        def load_x_quarter(q):
            lo, hi = q * QHW, (q + 1) * QHW
            for u in range(2):
                nc.sync.dma_start(
                    x_all[:, u * XH + lo * B:u * XH + hi * B],
                    x_d[u, :, lo:hi, :])

        load_x_quarter(0)

        # dim_red accumulator: four 32-partition blocks (one per PE column
        # group) accumulate concurrently; merged after the main loop.
        dimred_ps = dr_pool.tile([128, N], F32, space="PSUM")
        # bias seeding matmul into block 0: dimred[b, m] = 1 * db[m]
        nc.tensor.matmul(
            dimred_ps[0:B, :], lhsT=onesb_sb, rhs=db_sb, start=True,
            stop=False, skip_group_check=True, tile_position=(0, 0),
        )

        with ExitStack() as mctx:
            wdp = mctx.enter_context(tc.tile_pool(name="wdp", bufs=6))
            ep = mctx.enter_context(tc.tile_pool(name="ep", bufs=4))
            abf = mctx.enter_context(tc.tile_pool(name="abf", bufs=3))
            rp = mctx.enter_context(tc.tile_pool(name="rp", bufs=3))
            yp = mctx.enter_context(tc.tile_pool(name="yp", bufs=4))
            zp = mctx.enter_context(tc.tile_pool(name="zp", bufs=3, space="PSUM"))
            sp = mctx.enter_context(tc.tile_pool(name="sp", bufs=2, space="PSUM"))
            abp = mctx.enter_context(tc.tile_pool(name="abp", bufs=2, space="PSUM"))

            def xw(s, u):
                return x_all[:, u * HWP * B + s * SW * B:
                             u * HWP * B + (s + 1) * SW * B]

            # Software-pipelined emission: PE sees z MMs of slice s+2 and
            # ssum of s+1 between dependent ops of slice s, so it never
            # stalls in-order on the cross-engine softmax chain.
            state = {}

            def stage_z(s):
                # one 1.8MB wd DMA per slice, prefetched 2 slices ahead;
                # remaining x quarters ride between the first wd slices
                wd_sb = wdp.tile([128, 2 * SW * N], BF16, tag="wd",
                                 name=f"wd_{s}")
                nc.sync.dma_start(
                    wd_sb.rearrange("p (u t m) -> p u (t m)", u=2, t=SW),
                    wd_d[:, :, s * SW:(s + 1) * SW, :])
                if 1 <= s <= 3:
                    load_x_quarter(s)
                z_ps = zp.tile([K, SLW], F32, tag="z", space="PSUM",
                               name=f"z_{s}")
                nc.tensor.matmul(z_ps, lhsT=cw_sb[:, 0:K], rhs=xw(s, 0),
                                 start=True, stop=False)
                nc.tensor.matmul(z_ps, lhsT=cw_sb[:, K:2 * K], rhs=xw(s, 1),
                                 start=False, stop=True)
                e_sb = ep.tile([K, SLW], F32, tag="e", name=f"e_{s}")
                nc.scalar.activation(e_sb, z_ps, AF.Exp)
                # bf16 exp copy: lets the softmax-sum matmul run at 1 cyc/row
                eb_sb = ep.tile([K, SLW], BF16, tag="eb", name=f"eb_{s}")
                nc.scalar.activation(eb_sb, z_ps, AF.Exp)
                state[s] = (wd_sb, e_sb, eb_sb)

            def stage_mid(s):
                wd_sb, e_sb, eb_sb = state[s]
                ssum_ps = sp.tile([K, SLW], F32, tag="ss", space="PSUM",
                                  name=f"ss_{s}")
                nc.tensor.matmul(ssum_ps, lhsT=ones8_sb, rhs=eb_sb,
                                 start=True, stop=True)
                r_sb = rp.tile([K, SLW], F32, tag="r", name=f"r_{s}")
                nc.vector.reciprocal_approx_fast(r_sb, ssum_ps)
                ahw = ah_full[:, s * SLW:(s + 1) * SLW]
                nc.vector.tensor_tensor(out=ahw, in0=e_sb, in1=r_sb,
                                        op=ALU.mult)
                # bf16 copy (on ACT) so the broadcast matmul is 1 cyc/row
                ah_bf = abf.tile([K, SLW], BF16, tag="ahbf", name=f"abf_{s}")
                nc.scalar.copy(ah_bf, ahw)
                state[s] = (wd_sb, ah_bf)

            def stage_heavy(s):
                wd_sb, ah_bf = state.pop(s)
                ahb_ps = abp.tile([128, SLW], F32, tag="ab", space="PSUM",
                                  name=f"ab_{s}")
                nc.tensor.matmul(ahb_ps, lhsT=selk_sb, rhs=ah_bf,
                                 start=True, stop=True)
                for nh in range(2):
                    y_sb = yp.tile([128, SLW], BF16, tag="y",
                                   name=f"y_{s}_{nh}")
                    nc.vector.tensor_tensor(out=y_sb, in0=xw(s, nh),
                                            in1=ahb_ps, op=ALU.mult)
                    for j in range(SW):
                        g = (s * 2 + nh) * SW + j     # global chunk ordinal
                        blk = g % 4                   # PE column group
                        nc.tensor.matmul(
                            dimred_ps[blk * B:(blk + 1) * B, :],
                            lhsT=y_sb[:, j * B:(j + 1) * B],
                            rhs=wd_sb[:, (nh * SW + j) * N:
                                      (nh * SW + j + 1) * N],
                            start=(blk > 0 and g == blk),
                            stop=(g >= 2 * NSL * SW - 4),
                            skip_group_check=True,
                            tile_position=(0, blk * B),
                        )

            LEAD = 2
            for s in range(NSL + LEAD):
                if s < NSL:
                    stage_z(s)
                if 1 <= s and s - 1 < NSL:
                    stage_mid(s - 1)
                if s >= LEAD:
                    stage_heavy(s - LEAD)

        # ---------------- tail ----------------
        with ExitStack() as tctx:
            tp = tctx.enter_context(tc.tile_pool(name="tail_sb", bufs=1))
            tpp = tctx.enter_context(
                tc.tile_pool(name="tail_ps", bufs=1, space="PSUM")
            )
            hp = tctx.enter_context(
                tc.tile_pool(name="hyp_ps", bufs=1, space="PSUM")
            )
            hs = tctx.enter_context(tc.tile_pool(name="hyp_sb", bufs=2))

            # merge the 4 column-group blocks -> dim_red [b, m] f32 in SBUF:
            # one full-width PSUM->SBUF copy, then a selection-matrix matmul
            # sums partitions {b, 32+b, 64+b, 96+b} into row b.
            drg_sb = tp.tile([128, N], F32)
            nc.vector.tensor_copy(drg_sb, dimred_ps)
            drm_ps = tpp.tile([B, N], F32, tag="drm", space="PSUM")
            nc.tensor.matmul(drm_ps, lhsT=sel4_sb, rhs=drg_sb,
                             start=True, stop=True)
            dr_sb = tp.tile([B, N], F32)
            nc.vector.tensor_copy(dr_sb, drm_ps)
            drT_sb = tp.tile([128, 2 * B], F32)
            for nh in range(2):
                drT_ps = tpp.tile([128, B], F32, tag="drT", space="PSUM")
                nc.tensor.transpose(
                    drT_ps, dr_sb[:, nh * 128:(nh + 1) * 128], id32_sb
                )
                nc.vector.tensor_copy(drT_sb[:, nh * B:(nh + 1) * B], drT_ps)

            # hyp
            for c in range(CP // 128):
                hyp_ps = hp.tile([128, B], F32, tag="hyp", space="PSUM")
                nc.tensor.matmul(
                    hyp_ps, lhsT=wo_sb[:, c * 128:(c + 1) * 128],
                    rhs=drT_sb[:, 0:B], start=True, stop=False,
                )
                nc.tensor.matmul(
                    hyp_ps, lhsT=wo_sb[:, CP + c * 128:CP + (c + 1) * 128],
                    rhs=drT_sb[:, B:2 * B], start=False, stop=True,
                )
                hyp_sb = hs.tile([128, B], F32, tag="hyps")
                nc.scalar.activation(hyp_sb, hyp_ps, AF.Identity,
                                     bias=wob_sb[:, c:c + 1])
                nc.sync.dma_start(hyp_d[c], hyp_sb)

            # conf
            conf_ps = tpp.tile([1, B], F32, tag="conf", space="PSUM")
            nc.tensor.matmul(conf_ps, lhsT=wg_sb[:, 0:1], rhs=drT_sb[:, 0:B],
                             start=True, stop=False)
            nc.tensor.matmul(conf_ps, lhsT=wg_sb[:, 1:2], rhs=drT_sb[:, B:2 * B],
                             start=False, stop=True)
            conf_sb = tp.tile([1, B], F32)
            nc.scalar.activation(conf_sb, conf_ps, AF.Tanh, bias=wgb_sb[:, 0:1])
            nc.sync.dma_start(conf_d, conf_sb)

            # ---- loss ----
            # regather ah (hw < 196 region) to [(k h), (w b)] via DRAM bounce
            # (gpsimd ring: runs as soon as ah_full completes, not behind wd)
            ah_dram = dram.tile([K, H * W * B], F32, space="DRAM")
            nc.gpsimd.dma_start(ah_dram, ah_full[:, 0:H * W * B])
            A2 = tp.tile([K * H, W * B], F32)
            nc.gpsimd.dma_start(
                A2, ah_dram.rearrange("k (h rest) -> (k h) rest", h=H)
            )
            A2v = A2.rearrange("p (w b) -> p w b", b=B)
            G_ps = tpp.tile([W, B * W], F32, tag="G", space="PSUM")
            for b in range(B):
                ab = A2v[:, :, b:b + 1]
                nc.tensor.matmul(G_ps[:, b * W:(b + 1) * W], lhsT=ab, rhs=ab,
                                 start=True, stop=True, skip_group_check=True)
            S_ps = tpp.tile([K, W * B], F32, tag="S", space="PSUM")
            nc.tensor.matmul(S_ps, lhsT=hsel_sb, rhs=A2, start=True, stop=True)
            Gsq = tp.tile([W, B * W], F32)
            nc.scalar.activation(Gsq, G_ps, AF.Square)
            Ssq = tp.tile([K, W * B], F32)
            nc.scalar.activation(Ssq, S_ps, AF.Square)
            Gred = tp.tile([W, B], F32)
            nc.vector.tensor_reduce(
                Gred, Gsq.rearrange("p (b v) -> p b v", b=B),
                axis=AX.X, op=ALU.add,
            )
            Sred = tp.tile([K, B], F32)
            nc.vector.tensor_reduce(
                Sred, Ssq.rearrange("p (w b) -> p b w", b=B),
                axis=AX.X, op=ALU.add,
            )
            l_ps = tpp.tile([1, B], F32, tag="l", space="PSUM")
            nc.tensor.matmul(l_ps, lhsT=ones14_sb, rhs=Gred,
                             start=True, stop=False)
            nc.tensor.matmul(l_ps, lhsT=mones8_sb, rhs=Sred,
                             start=False, stop=True)
            loss_sb = tp.tile([1, B], F32)
            nc.vector.tensor_copy(loss_sb, l_ps)
            nc.sync.dma_start(loss_d, loss_sb)

    return nc


def _bf16(a):
    return np.ascontiguousarray(a.astype(ml_dtypes.bfloat16))


def build_host_inputs(x, conv_w, dimred_w, dimred_b, Wo_w, Wo_b, Wg_w, Wg_b):
    """Returns in_maps: one dict per core."""
    x = np.asarray(x, np.float32)
    conv_w = np.asarray(conv_w, np.float32)
    dimred_w = np.asarray(dimred_w, np.float32)
    dimred_b = np.asarray(dimred_b, np.float32)
    Wo_w = np.asarray(Wo_w, np.float32)
    Wo_b = np.asarray(Wo_b, np.float32)
    Wg_w = np.asarray(Wg_w, np.float32)
    Wg_b = np.asarray(Wg_b, np.float32)

    # x_t[nh, nl, hw, b] = x[b, nh*128+nl, hw]  (partition-major contiguous)
    xt = x.transpose(1, 2, 3, 0).reshape(N, HW, B)          # [n, hw, b]
    xt = _bf16(xt.reshape(2, 128, HWP, B))

    # conv_w^T [2, 128, K]
    cwt = _bf16(conv_w.T.reshape(2, 128, K))

    shared = {
        "x_t": xt,
        "cw_t": cwt,
        "ones8": _bf16(np.ones((8, 8), np.float32)),
        "hsel": np.repeat(np.eye(K, dtype=np.float32), H, axis=0),
        "ones14": np.ones((14, 1), np.float32),
        "mones8": -np.ones((8, 1), np.float32),
        "onesb": np.ones((1, B), np.float32),
        "id32": np.eye(32, dtype=np.float32),
        "sel4": np.tile(np.eye(B, dtype=np.float32), (4, 1)),
    }

    in_maps = []
    for k in range(NCORES):
        # wd_t[nl, nh, hw, m] = dimred_w[k, m, n, hw]  (partition-outermost)
        wd = dimred_w[k].reshape(N, N, HW).transpose(1, 2, 0)   # [n, hw, m]
        wd = wd.reshape(2, 128, HWP, N).transpose(1, 0, 2, 3)   # [128, 2, hw, m]
        wo = np.zeros((CP, N), np.float32)
        wo[:C] = Wo_w[k]
        wob = np.zeros((CP,), np.float32)
        wob[:C] = Wo_b[k]
        selk = np.zeros((8, 128), np.float32)
        selk[k] = 1.0
        m = dict(shared)
        m.update({
            "wd_t": _bf16(wd),
            "wo_t": np.ascontiguousarray(wo.T.reshape(2, 128, CP)),
            "wob_t": wob.reshape(CP // 128, 128),
            "wg_t": np.ascontiguousarray(Wg_w[k].reshape(2, 128, 1)),
            "wgb_t": np.full((1, 1), Wg_b[k], np.float32),
            "db_t": dimred_b[k].reshape(1, N),
            "selk": _bf16(selk),
        })
        in_maps.append(m)
    return in_maps


def assemble_outputs(results):
    hyp = np.stack(
        [r["out_hyp"].reshape(CP, B)[:C].T for r in results], axis=1
    )                                                   # [B, K, C]
    conf = np.stack([r["out_conf"][0] for r in results], axis=1)[..., None]
    loss = results[0]["out_loss"][0][:, None]           # [B, 1]
    return (
        np.ascontiguousarray(hyp, np.float32),
        np.ascontiguousarray(conf, np.float32),
        np.ascontiguousarray(loss, np.float32),
    )


_GRAPH_CACHE = {}


def get_graph():
    if "nc" not in _GRAPH_CACHE:
        nc = build_graph()
        nc.finalize()
        _GRAPH_CACHE["nc"] = nc
    return _GRAPH_CACHE["nc"]


def kernel(**inputs):
    nc = get_graph()
    in_maps = build_host_inputs(**inputs)
    res = run_bass_kernel_spmd(nc, in_maps, core_ids=list(range(NCORES)))
    return assemble_outputs(res.results)


# revision 65
# speedup vs baseline: 1.0848x; 1.0268x over previous
"""Trainium2 Bass kernel for nn_Attention_9844065042780.

Sharding: expert-parallel over the K=8 independent groups, one group per
NeuronCore (8 cores).  Each core receives the full activations x (reordered
host-side), the full conv_w (to compute the shared softmax attention maps
and the shared orthogonality loss), and only its own group's
dimred/Wo/Wg weights.  Outputs are disjoint per-core slices (hyp[:,k,:],
conf[:,k]) plus the (identical on every core) loss, gathered host-side.

Per-core math (k = this core's group):
  z[k', hw, b]   = sum_n conv_w[k', n] x[b, n, hw]            (PE)
  ah[k', hw, b]  = softmax_k'(z)                              (ACT exp, PE sum, DVE recip/mul)
  y[(hw,n), b]   = ah[k, hw, b] * x[b, n, hw]                 (PE row-broadcast + DVE mul)
  dim_red[b, m]  = sum_{hw,n} y * wd[m, n, hw] + db[m]        (PE, 417 accumulating matmuls)
  hyp[c, b]      = sum_n Wo[c, n] dim_red[b, n] + Wo_b[c]     (PE + ACT bias)
  conf[b]        = tanh(sum_n Wg[n] dim_red[b, n] + Wg_b)     (PE + ACT)
  loss[b]        = ||A_b^T A_b||_F^2 - sum((H^T A_b)^2),  A_b = ah[:, :, :, b]  (PE/ACT/DVE)

Layouts (host-prepared, hw padded 196->208 so slices are uniform 16*32=512):
  x_t  [2, 208, 128, 32]  bf16   x_t[nh, hw, nl, b] = x[b, nh*128+nl, hw]
  wd_t [2, 208, 128, 256] bf16   wd_t[nh, hw, nl, m] = dimred_w[k, m, nh*128+nl, hw]
"""

import os
import numpy as np
import ml_dtypes
from contextlib import ExitStack

from concourse import bass, bacc, tile, mybir
from concourse.bass_utils import run_bass_kernel_spmd

F32 = mybir.dt.float32
BF16 = mybir.dt.bfloat16
AF = mybir.ActivationFunctionType
ALU = mybir.AluOpType
AX = mybir.AxisListType

B, N, H, W, K, C = 32, 256, 14, 14, 8, 1000
HW, HWP, CP = 196, 196, 1024
SW = 14                      # hw positions per slice (one h row)
NSL = HWP // SW              # 14 slices
SLW = SW * B                 # 448 free columns per slice
NCORES = 8


def build_graph():
    nc = bacc.Bacc("TRN2", target_bir_lowering=False, debug=False)

    def inp(name, shape, dtype):
        return nc.dram_tensor(name, shape, dtype, kind="ExternalInput").ap()

    def outp(name, shape, dtype):
        return nc.dram_tensor(name, shape, dtype, kind="ExternalOutput").ap()

    x_d = inp("x_t", [2, 128, HWP, B], BF16)
    wd_d = inp("wd_t", [128, 2, HWP, N], BF16)
    cw_d = inp("cw_t", [2, 128, K], BF16)
    wo_d = inp("wo_t", [2, 128, CP], F32)
    wob_d = inp("wob_t", [CP // 128, 128], F32)
    wg_d = inp("wg_t", [2, 128, 1], F32)
    wgb_d = inp("wgb_t", [1, 1], F32)
    db_d = inp("db_t", [1, N], F32)
    ones8_d = inp("ones8", [8, 8], BF16)
    selk_d = inp("selk", [8, 128], BF16)
    hsel_d = inp("hsel", [K * H, K], F32)
    ones14_d = inp("ones14", [14, 1], F32)
    mones8_d = inp("mones8", [8, 1], F32)
    onesb_d = inp("onesb", [1, B], F32)
    id32_d = inp("id32", [32, 32], F32)
    sel4_d = inp("sel4", [128, B], F32)

    hyp_d = outp("out_hyp", [CP // 128, 128, B], F32)
    conf_d = outp("out_conf", [1, B], F32)
    loss_d = outp("out_loss", [1, B], F32)

    with tile.TileContext(nc) as tc, ExitStack() as ctx:
        const = ctx.enter_context(tc.tile_pool(name="const", bufs=1))
        persist = ctx.enter_context(tc.tile_pool(name="persist", bufs=1))
        dram = ctx.enter_context(tc.tile_pool(name="dram", bufs=1, space="DRAM"))
        dr_pool = ctx.enter_context(
            tc.tile_pool(name="dr_psum", bufs=1, space="PSUM")
        )

        # PE warm-up first in program order: memset is gpsimd's first
        # instruction and the matmuls open the HAM clock gate
        # (1.2 -> 2.4 GHz) while the DMA pipeline ramps.
        with ExitStack() as wctx:
            wup = wctx.enter_context(tc.tile_pool(name="wup", bufs=1))
            wupp = wctx.enter_context(
                tc.tile_pool(name="wupp", bufs=1, space="PSUM"))
            wu_sb = wup.tile([128, 512], BF16)
            nc.gpsimd.memset(wu_sb, 0.0)
            wu_ps = wupp.tile([128, 512], F32, space="PSUM")
            for _ in range(12):
                nc.tensor.matmul(wu_ps, lhsT=wu_sb[:, 0:128], rhs=wu_sb,
                                 start=True, stop=True)

        # ---- constants into SBUF ----
        # Main-loop-critical consts go FIRST on the sync HWDGE ring (tiny);
        # tail-only consts go on the gpsimd SWDGE ring so neither the sync
        # ring (wd stream) nor the scalar engine (exp/copies) is blocked.
        cw_sb = const.tile([128, 2 * K], BF16)
        nc.sync.dma_start(cw_sb.rearrange("p (t c) -> p t c", t=2),
                          cw_d.rearrange("t p c -> p t c"))
        db_sb = const.tile([1, N], F32)
        nc.sync.dma_start(db_sb, db_d)
        ones8_sb = const.tile([8, 8], BF16)
        nc.sync.dma_start(ones8_sb, ones8_d)
        selk_sb = const.tile([8, 128], BF16)
        nc.sync.dma_start(selk_sb, selk_d)
        onesb_sb = const.tile([1, B], F32)
        nc.sync.dma_start(onesb_sb, onesb_d)

        wo_sb = const.tile([128, 2 * CP], F32)
        nc.gpsimd.dma_start(wo_sb.rearrange("p (t c) -> p t c", t=2),
                            wo_d.rearrange("t p c -> p t c"))
        wob_sb = const.tile([128, CP // 128], F32)
        nc.gpsimd.dma_start(wob_sb, wob_d.rearrange("c p -> p c"))
        wg_sb = const.tile([128, 2], F32)
        nc.gpsimd.dma_start(wg_sb.rearrange("p (t one) -> p t one", t=2),
                            wg_d.rearrange("t p one -> p t one"))
        wgb_sb = const.tile([1, 1], F32)
        nc.gpsimd.dma_start(wgb_sb, wgb_d)
        hsel_sb = const.tile([K * H, K], F32)
        nc.gpsimd.dma_start(hsel_sb, hsel_d)
        ones14_sb = const.tile([14, 1], F32)
        nc.gpsimd.dma_start(ones14_sb, ones14_d)
        mones8_sb = const.tile([8, 1], F32)
        nc.gpsimd.dma_start(mones8_sb, mones8_d)
        id32_sb = const.tile([32, 32], F32)
        nc.gpsimd.dma_start(id32_sb, id32_d)
        sel4_sb = const.tile([128, B], F32)
        nc.gpsimd.dma_start(sel4_sb, sel4_d)

        # full attention map, all K groups: [8, (hw, b)] fp32
        ah_full = persist.tile([K, HWP * B], F32)

        # x resident in SBUF: [128, (u, hw, b)].  Loaded on the sync ring in
        # quarter-DMAs interleaved with the first wd slices (emitted in
        # stage_z below), first quarter up-front so slice 0 unblocks early.
        x_all = persist.tile([128, 2 * HWP * B], BF16)
        XH = HWP * B
        QHW = 49  # hw positions per x quarter-load (49*4 = 196)

        def load_x_quarter(q):
            lo, hi = q * QHW, (q + 1) * QHW
            for u in range(2):
                nc.sync.dma_start(
                    x_all[:, u * XH + lo * B:u * XH + hi * B],
                    x_d[u, :, lo:hi, :])

        load_x_quarter(0)

        # dim_red accumulator: four 32-partition blocks (one per PE column
        # group) accumulate concurrently; merged after the main loop.
        dimred_ps = dr_pool.tile([128, N], F32, space="PSUM")
        # bias seeding matmul into block 0: dimred[b, m] = 1 * db[m]
        nc.tensor.matmul(
            dimred_ps[0:B, :], lhsT=onesb_sb, rhs=db_sb, start=True,
            stop=False, skip_group_check=True, tile_position=(0, 0),
        )

        with ExitStack() as mctx:
            wdp = mctx.enter_context(tc.tile_pool(name="wdp", bufs=6))
            ep = mctx.enter_context(tc.tile_pool(name="ep", bufs=4))
            abf = mctx.enter_context(tc.tile_pool(name="abf", bufs=3))
            rp = mctx.enter_context(tc.tile_pool(name="rp", bufs=3))
            yp = mctx.enter_context(tc.tile_pool(name="yp", bufs=4))
            zp = mctx.enter_context(tc.tile_pool(name="zp", bufs=2, space="PSUM"))
            sp = mctx.enter_context(tc.tile_pool(name="sp", bufs=1, space="PSUM"))
            abp = mctx.enter_context(tc.tile_pool(name="abp", bufs=1, space="PSUM"))
            lp = mctx.enter_context(tc.tile_pool(name="lp", bufs=1, space="PSUM"))
            lsb = mctx.enter_context(tc.tile_pool(name="lsb", bufs=1))

            def xw(s, u):
                return x_all[:, u * HWP * B + s * SW * B:
                             u * HWP * B + (s + 1) * SW * B]

            # Software-pipelined emission: PE sees z MMs of slice s+2 and
            # ssum of s+1 between dependent ops of slice s, so it never
            # stalls in-order on the cross-engine softmax chain.
            state = {}

            def stage_z(s):
                # one 1.8MB wd DMA per slice, prefetched 2 slices ahead;
                # remaining x quarters ride between the first wd slices
                wd_sb = wdp.tile([128, 2 * SW * N], BF16, tag="wd",
                                 name=f"wd_{s}")
                nc.sync.dma_start(
                    wd_sb.rearrange("p (u t m) -> p u (t m)", u=2, t=SW),
                    wd_d[:, :, s * SW:(s + 1) * SW, :])
                if 1 <= s <= 3:
                    load_x_quarter(s)
                z_ps = zp.tile([K, SLW], F32, tag="z", space="PSUM",
                               name=f"z_{s}")
                nc.tensor.matmul(z_ps, lhsT=cw_sb[:, 0:K], rhs=xw(s, 0),
                                 start=True, stop=False)
                nc.tensor.matmul(z_ps, lhsT=cw_sb[:, K:2 * K], rhs=xw(s, 1),
                                 start=False, stop=True)
                e_sb = ep.tile([K, SLW], F32, tag="e", name=f"e_{s}")
                nc.scalar.activation(e_sb, z_ps, AF.Exp)
                # bf16 exp copy: lets the softmax-sum matmul run at 1 cyc/row
                eb_sb = ep.tile([K, SLW], BF16, tag="eb", name=f"eb_{s}")
                nc.scalar.activation(eb_sb, z_ps, AF.Exp)
                state[s] = (wd_sb, e_sb, eb_sb)

            def stage_mid(s):
                wd_sb, e_sb, eb_sb = state[s]
                ssum_ps = sp.tile([K, SLW], F32, tag="ss", space="PSUM",
                                  name=f"ss_{s}")
                nc.tensor.matmul(ssum_ps, lhsT=ones8_sb, rhs=eb_sb,
                                 start=True, stop=True)
                r_sb = rp.tile([K, SLW], F32, tag="r", name=f"r_{s}")
                nc.vector.reciprocal_approx_fast(r_sb, ssum_ps)
                ahw = ah_full[:, s * SLW:(s + 1) * SLW]
                nc.vector.tensor_tensor(out=ahw, in0=e_sb, in1=r_sb,
                                        op=ALU.mult)
                # bf16 copy (on ACT) so the broadcast matmul is 1 cyc/row
                ah_bf = abf.tile([K, SLW], BF16, tag="ahbf", name=f"abf_{s}")
                nc.scalar.copy(ah_bf, ahw)
                state[s] = (wd_sb, ah_bf)

            def stage_heavy(s):
                wd_sb, ah_bf = state.pop(s)
                ahb_ps = abp.tile([128, SLW], F32, tag="ab", space="PSUM",
                                  name=f"ab_{s}")
                nc.tensor.matmul(ahb_ps, lhsT=selk_sb, rhs=ah_bf,
                                 start=True, stop=True)
                for nh in range(2):
                    y_sb = yp.tile([128, SLW], BF16, tag="y",
                                   name=f"y_{s}_{nh}")
                    nc.vector.tensor_tensor(out=y_sb, in0=xw(s, nh),
                                            in1=ahb_ps, op=ALU.mult)
                    for j in range(SW):
                        g = (s * 2 + nh) * SW + j     # global chunk ordinal
                        blk = g % 4                   # PE column group
                        nc.tensor.matmul(
                            dimred_ps[blk * B:(blk + 1) * B, :],
                            lhsT=y_sb[:, j * B:(j + 1) * B],
                            rhs=wd_sb[:, (nh * SW + j) * N:
                                      (nh * SW + j + 1) * N],
                            start=(blk > 0 and g == blk),
                            stop=(g >= 2 * NSL * SW - 4),
                            skip_group_check=True,
                            tile_position=(0, blk * B),
                        )


            def emit_loss():
                # ---- loss (overlaps the tail of the wd stream) ----
                # regather ah to [(k h), (w b)] via DRAM bounce on gpsimd
                ah_dram = dram.tile([K, H * W * B], F32, space="DRAM")
                nc.gpsimd.dma_start(ah_dram, ah_full[:, 0:H * W * B])
                A2 = lsb.tile([K * H, W * B], F32)
                nc.gpsimd.dma_start(
                    A2, ah_dram.rearrange("k (h rest) -> (k h) rest", h=H)
                )
                A2v = A2.rearrange("p (w b) -> p w b", b=B)
                G_ps = lp.tile([W, B * W], F32, tag="G", space="PSUM")
                for b in range(B):
                    ab = A2v[:, :, b:b + 1]
                    nc.tensor.matmul(G_ps[:, b * W:(b + 1) * W], lhsT=ab,
                                     rhs=ab, start=True, stop=True,
                                     skip_group_check=True)
                S_ps = lp.tile([K, W * B], F32, tag="S", space="PSUM")
                nc.tensor.matmul(S_ps, lhsT=hsel_sb, rhs=A2,
                                 start=True, stop=True)
                Gsq = lsb.tile([W, B * W], F32)
                nc.scalar.activation(Gsq, G_ps, AF.Square)
                Ssq = lsb.tile([K, W * B], F32)
                nc.scalar.activation(Ssq, S_ps, AF.Square)
                Gred = lsb.tile([W, B], F32)
                nc.vector.tensor_reduce(
                    Gred, Gsq.rearrange("p (b v) -> p b v", b=B),
                    axis=AX.X, op=ALU.add,
                )
                Sred = lsb.tile([K, B], F32)
                nc.vector.tensor_reduce(
                    Sred, Ssq.rearrange("p (w b) -> p b w", b=B),
                    axis=AX.X, op=ALU.add,
                )
                l_ps = lp.tile([1, B], F32, tag="l", space="PSUM")
                nc.tensor.matmul(l_ps, lhsT=ones14_sb, rhs=Gred,
                                 start=True, stop=False)
                nc.tensor.matmul(l_ps, lhsT=mones8_sb, rhs=Sred,
                                 start=False, stop=True)
                loss_sb = lsb.tile([1, B], F32)
                nc.vector.tensor_copy(loss_sb, l_ps)
                nc.gpsimd.dma_start(loss_d, loss_sb)

            LEAD = 2
            for s in range(NSL + LEAD):
                if s < NSL:
                    stage_z(s)
                if 1 <= s and s - 1 < NSL:
                    stage_mid(s - 1)
                if s == NSL:
                    emit_loss()
                if s >= LEAD:
                    stage_heavy(s - LEAD)

        # ---------------- tail ----------------
        with ExitStack() as tctx:
            tp = tctx.enter_context(tc.tile_pool(name="tail_sb", bufs=1))
            tpp = tctx.enter_context(
                tc.tile_pool(name="tail_ps", bufs=1, space="PSUM")
            )
            hp = tctx.enter_context(
                tc.tile_pool(name="hyp_ps", bufs=3, space="PSUM")
            )
            hs = tctx.enter_context(tc.tile_pool(name="hyp_sb", bufs=2))

            # merge the 4 column-group blocks -> dim_red [b, m] f32 in SBUF:
            # one full-width PSUM->SBUF copy, then a selection-matrix matmul
            # sums partitions {b, 32+b, 64+b, 96+b} into row b.
            drg_sb = tp.tile([128, N], F32)
            nc.vector.tensor_copy(drg_sb, dimred_ps)
            drm_ps = tpp.tile([B, N], F32, tag="drm", space="PSUM")
            nc.tensor.matmul(drm_ps, lhsT=sel4_sb, rhs=drg_sb,
                             start=True, stop=True)
            dr_sb = tp.tile([B, N], F32)
            nc.vector.tensor_copy(dr_sb, drm_ps)
            drT_sb = tp.tile([128, 2 * B], F32)
            for nh in range(2):
                drT_ps = tpp.tile([128, B], F32, tag="drT", space="PSUM")
                nc.tensor.transpose(
                    drT_ps, dr_sb[:, nh * 128:(nh + 1) * 128], id32_sb
                )
                nc.vector.tensor_copy(drT_sb[:, nh * B:(nh + 1) * B], drT_ps)

            # hyp
            for c in range(CP // 128):
                hyp_ps = hp.tile([128, B], F32, tag="hyp", space="PSUM")
                nc.tensor.matmul(
                    hyp_ps, lhsT=wo_sb[:, c * 128:(c + 1) * 128],
                    rhs=drT_sb[:, 0:B], start=True, stop=False,
                )
                nc.tensor.matmul(
                    hyp_ps, lhsT=wo_sb[:, CP + c * 128:CP + (c + 1) * 128],
                    rhs=drT_sb[:, B:2 * B], start=False, stop=True,
                )
                hyp_sb = hs.tile([128, B], F32, tag="hyps")
                nc.scalar.activation(hyp_sb, hyp_ps, AF.Identity,
                                     bias=wob_sb[:, c:c + 1])
                nc.sync.dma_start(hyp_d[c], hyp_sb)

            # conf
            conf_ps = tpp.tile([1, B], F32, tag="conf", space="PSUM")
            nc.tensor.matmul(conf_ps, lhsT=wg_sb[:, 0:1], rhs=drT_sb[:, 0:B],
                             start=True, stop=False)
            nc.tensor.matmul(conf_ps, lhsT=wg_sb[:, 1:2], rhs=drT_sb[:, B:2 * B],
                             start=False, stop=True)
            conf_sb = tp.tile([1, B], F32)
            nc.scalar.activation(conf_sb, conf_ps, AF.Tanh, bias=wgb_sb[:, 0:1])
            nc.sync.dma_start(conf_d, conf_sb)


    return nc


def _bf16(a):
    return np.ascontiguousarray(a.astype(ml_dtypes.bfloat16))


def build_host_inputs(x, conv_w, dimred_w, dimred_b, Wo_w, Wo_b, Wg_w, Wg_b):
    """Returns in_maps: one dict per core."""
    x = np.asarray(x, np.float32)
    conv_w = np.asarray(conv_w, np.float32)
    dimred_w = np.asarray(dimred_w, np.float32)
    dimred_b = np.asarray(dimred_b, np.float32)
    Wo_w = np.asarray(Wo_w, np.float32)
    Wo_b = np.asarray(Wo_b, np.float32)
    Wg_w = np.asarray(Wg_w, np.float32)
    Wg_b = np.asarray(Wg_b, np.float32)

    # x_t[nh, nl, hw, b] = x[b, nh*128+nl, hw]  (partition-major contiguous)
    xt = x.transpose(1, 2, 3, 0).reshape(N, HW, B)          # [n, hw, b]
    xt = _bf16(xt.reshape(2, 128, HWP, B))

    # conv_w^T [2, 128, K]
    cwt = _bf16(conv_w.T.reshape(2, 128, K))

    shared = {
        "x_t": xt,
        "cw_t": cwt,
        "ones8": _bf16(np.ones((8, 8), np.float32)),
        "hsel": np.repeat(np.eye(K, dtype=np.float32), H, axis=0),
        "ones14": np.ones((14, 1), np.float32),
        "mones8": -np.ones((8, 1), np.float32),
        "onesb": np.ones((1, B), np.float32),
        "id32": np.eye(32, dtype=np.float32),
        "sel4": np.tile(np.eye(B, dtype=np.float32), (4, 1)),
    }

    in_maps = []
    for k in range(NCORES):
        # wd_t[nl, nh, hw, m] = dimred_w[k, m, n, hw]  (partition-outermost)
        wd = dimred_w[k].reshape(N, N, HW).transpose(1, 2, 0)   # [n, hw, m]
        wd = wd.reshape(2, 128, HWP, N).transpose(1, 0, 2, 3)   # [128, 2, hw, m]
        wo = np.zeros((CP, N), np.float32)
        wo[:C] = Wo_w[k]
        wob = np.zeros((CP,), np.float32)
        wob[:C] = Wo_b[k]
        selk = np.zeros((8, 128), np.float32)
        selk[k] = 1.0
        m = dict(shared)
        m.update({
            "wd_t": _bf16(wd),
            "wo_t": np.ascontiguousarray(wo.T.reshape(2, 128, CP)),
            "wob_t": wob.reshape(CP // 128, 128),
            "wg_t": np.ascontiguousarray(Wg_w[k].reshape(2, 128, 1)),
            "wgb_t": np.full((1, 1), Wg_b[k], np.float32),
            "db_t": dimred_b[k].reshape(1, N),
            "selk": _bf16(selk),
        })
        in_maps.append(m)
    return in_maps


def assemble_outputs(results):
    hyp = np.stack(
        [r["out_hyp"].reshape(CP, B)[:C].T for r in results], axis=1
    )                                                   # [B, K, C]
    conf = np.stack([r["out_conf"][0] for r in results], axis=1)[..., None]
    loss = results[0]["out_loss"][0][:, None]           # [B, 1]
    return (
        np.ascontiguousarray(hyp, np.float32),
        np.ascontiguousarray(conf, np.float32),
        np.ascontiguousarray(loss, np.float32),
    )


_GRAPH_CACHE = {}


def get_graph():
    if "nc" not in _GRAPH_CACHE:
        nc = build_graph()
        nc.finalize()
        _GRAPH_CACHE["nc"] = nc
    return _GRAPH_CACHE["nc"]


def kernel(**inputs):
    nc = get_graph()
    in_maps = build_host_inputs(**inputs)
    res = run_bass_kernel_spmd(nc, in_maps, core_ids=list(range(NCORES)))
    return assemble_outputs(res.results)


# revision 66
# speedup vs baseline: 1.0925x; 1.0071x over previous
"""Trainium2 Bass kernel for nn_Attention_9844065042780.

Sharding: expert-parallel over the K=8 independent groups, one group per
NeuronCore (8 cores).  Each core receives the full activations x (reordered
host-side), the full conv_w (to compute the shared softmax attention maps
and the shared orthogonality loss), and only its own group's
dimred/Wo/Wg weights.  Outputs are disjoint per-core slices (hyp[:,k,:],
conf[:,k]) plus the (identical on every core) loss, gathered host-side.

Per-core math (k = this core's group):
  z[k', hw, b]   = sum_n conv_w[k', n] x[b, n, hw]            (PE)
  ah[k', hw, b]  = softmax_k'(z)                              (ACT exp, PE sum, DVE recip/mul)
  y[(hw,n), b]   = ah[k, hw, b] * x[b, n, hw]                 (PE row-broadcast + DVE mul)
  dim_red[b, m]  = sum_{hw,n} y * wd[m, n, hw] + db[m]        (PE, 417 accumulating matmuls)
  hyp[c, b]      = sum_n Wo[c, n] dim_red[b, n] + Wo_b[c]     (PE + ACT bias)
  conf[b]        = tanh(sum_n Wg[n] dim_red[b, n] + Wg_b)     (PE + ACT)
  loss[b]        = ||A_b^T A_b||_F^2 - sum((H^T A_b)^2),  A_b = ah[:, :, :, b]  (PE/ACT/DVE)

Layouts (host-prepared, hw padded 196->208 so slices are uniform 16*32=512):
  x_t  [2, 208, 128, 32]  bf16   x_t[nh, hw, nl, b] = x[b, nh*128+nl, hw]
  wd_t [2, 208, 128, 256] bf16   wd_t[nh, hw, nl, m] = dimred_w[k, m, nh*128+nl, hw]
"""

import os
import numpy as np
import ml_dtypes
from contextlib import ExitStack

from concourse import bass, bacc, tile, mybir
from concourse.bass_utils import run_bass_kernel_spmd

F32 = mybir.dt.float32
BF16 = mybir.dt.bfloat16
AF = mybir.ActivationFunctionType
ALU = mybir.AluOpType
AX = mybir.AxisListType

B, N, H, W, K, C = 32, 256, 14, 14, 8, 1000
HW, HWP, CP = 196, 196, 1024
SW = 14                      # hw positions per slice (one h row)
NSL = HWP // SW              # 14 slices
SLW = SW * B                 # 448 free columns per slice
NCORES = 8


def build_graph():
    nc = bacc.Bacc("TRN2", target_bir_lowering=False, debug=False)

    def inp(name, shape, dtype):
        return nc.dram_tensor(name, shape, dtype, kind="ExternalInput").ap()

    def outp(name, shape, dtype):
        return nc.dram_tensor(name, shape, dtype, kind="ExternalOutput").ap()

    x_d = inp("x_t", [2, 128, HWP, B], BF16)
    wd_d = inp("wd_t", [128, 2, HWP, N], BF16)
    cw_d = inp("cw_t", [2, 128, K], BF16)
    wo_d = inp("wo_t", [2, 128, CP], F32)
    wob_d = inp("wob_t", [CP // 128, 128], F32)
    wg_d = inp("wg_t", [2, 128, 1], F32)
    wgb_d = inp("wgb_t", [1, 1], F32)
    db_d = inp("db_t", [1, N], F32)
    ones8_d = inp("ones8", [8, 8], BF16)
    selk_d = inp("selk", [8, 128], BF16)
    hsel_d = inp("hsel", [K * H, K], F32)
    ones14_d = inp("ones14", [14, 1], F32)
    mones8_d = inp("mones8", [8, 1], F32)
    onesb_d = inp("onesb", [1, B], F32)
    sel4_d = inp("sel4", [128, B], F32)

    hyp_d = outp("out_hyp", [CP // 128, 128, B], F32)
    conf_d = outp("out_conf", [1, B], F32)
    loss_d = outp("out_loss", [1, B], F32)

    with tile.TileContext(nc) as tc, ExitStack() as ctx:
        const = ctx.enter_context(tc.tile_pool(name="const", bufs=1))
        persist = ctx.enter_context(tc.tile_pool(name="persist", bufs=1))
        dram = ctx.enter_context(tc.tile_pool(name="dram", bufs=1, space="DRAM"))
        dr_pool = ctx.enter_context(
            tc.tile_pool(name="dr_psum", bufs=1, space="PSUM")
        )

        # PE warm-up first in program order: memset is gpsimd's first
        # instruction and the matmuls open the HAM clock gate
        # (1.2 -> 2.4 GHz) while the DMA pipeline ramps.
        with ExitStack() as wctx:
            wup = wctx.enter_context(tc.tile_pool(name="wup", bufs=1))
            wupp = wctx.enter_context(
                tc.tile_pool(name="wupp", bufs=1, space="PSUM"))
            wu_sb = wup.tile([128, 512], BF16)
            nc.gpsimd.memset(wu_sb, 0.0)
            wu_ps = wupp.tile([128, 512], F32, space="PSUM")
            for _ in range(12):
                nc.tensor.matmul(wu_ps, lhsT=wu_sb[:, 0:128], rhs=wu_sb,
                                 start=True, stop=True)

        # ---- constants into SBUF ----
        # Main-loop-critical consts go FIRST on the sync HWDGE ring (tiny);
        # tail-only consts go on the gpsimd SWDGE ring so neither the sync
        # ring (wd stream) nor the scalar engine (exp/copies) is blocked.
        cw_sb = const.tile([128, 2 * K], BF16)
        nc.sync.dma_start(cw_sb.rearrange("p (t c) -> p t c", t=2),
                          cw_d.rearrange("t p c -> p t c"))
        db_sb = const.tile([1, N], F32)
        nc.sync.dma_start(db_sb, db_d)
        ones8_sb = const.tile([8, 8], BF16)
        nc.sync.dma_start(ones8_sb, ones8_d)
        selk_sb = const.tile([8, 128], BF16)
        nc.sync.dma_start(selk_sb, selk_d)
        onesb_sb = const.tile([1, B], F32)
        nc.sync.dma_start(onesb_sb, onesb_d)

        # full attention map, all K groups: [8, (hw, b)] fp32
        ah_full = persist.tile([K, HWP * B], F32)

        # x resident in SBUF: [128, (u, hw, b)].  Loaded on the sync ring in
        # quarter-DMAs interleaved with the first wd slices (emitted in
        # stage_z below), first quarter up-front so slice 0 unblocks early.
        x_all = persist.tile([128, 2 * HWP * B], BF16)
        XH = HWP * B
        QHW = 49  # hw positions per x quarter-load (49*4 = 196)

        def load_x_quarter(q):
            lo, hi = q * QHW, (q + 1) * QHW
            for u in range(2):
                nc.gpsimd.dma_start(
                    x_all[:, u * XH + lo * B:u * XH + hi * B],
                    x_d[u, :, lo:hi, :])

        for q in range(4):
            load_x_quarter(q)

        wo_sb = const.tile([128, 2 * CP], F32)
        nc.gpsimd.dma_start(wo_sb.rearrange("p (t c) -> p t c", t=2),
                            wo_d.rearrange("t p c -> p t c"))
        wob_sb = const.tile([128, CP // 128], F32)
        nc.gpsimd.dma_start(wob_sb, wob_d.rearrange("c p -> p c"))
        wg_sb = const.tile([128, 2], F32)
        nc.gpsimd.dma_start(wg_sb.rearrange("p (t one) -> p t one", t=2),
                            wg_d.rearrange("t p one -> p t one"))
        wgb_sb = const.tile([1, 1], F32)
        nc.gpsimd.dma_start(wgb_sb, wgb_d)
        hsel_sb = const.tile([K * H, K], F32)
        nc.gpsimd.dma_start(hsel_sb, hsel_d)
        ones14_sb = const.tile([14, 1], F32)
        nc.gpsimd.dma_start(ones14_sb, ones14_d)
        mones8_sb = const.tile([8, 1], F32)
        nc.gpsimd.dma_start(mones8_sb, mones8_d)
        sel4_sb = const.tile([128, B], F32)
        nc.gpsimd.dma_start(sel4_sb, sel4_d)

        # dim_red accumulator: four 32-partition blocks (one per PE column
        # group) accumulate concurrently; merged after the main loop.
        dimred_ps = dr_pool.tile([128, N], F32, space="PSUM")
        # bias seeding matmul into block 0: dimred[b, m] = 1 * db[m]
        nc.tensor.matmul(
            dimred_ps[0:B, :], lhsT=onesb_sb, rhs=db_sb, start=True,
            stop=False, skip_group_check=True, tile_position=(0, 0),
        )

        with ExitStack() as mctx:
            wdp = mctx.enter_context(tc.tile_pool(name="wdp", bufs=6))
            ep = mctx.enter_context(tc.tile_pool(name="ep", bufs=4))
            abf = mctx.enter_context(tc.tile_pool(name="abf", bufs=3))
            rp = mctx.enter_context(tc.tile_pool(name="rp", bufs=3))
            yp = mctx.enter_context(tc.tile_pool(name="yp", bufs=4))
            zp = mctx.enter_context(tc.tile_pool(name="zp", bufs=2, space="PSUM"))
            sp = mctx.enter_context(tc.tile_pool(name="sp", bufs=1, space="PSUM"))
            abp = mctx.enter_context(tc.tile_pool(name="abp", bufs=1, space="PSUM"))
            lp = mctx.enter_context(tc.tile_pool(name="lp", bufs=1, space="PSUM"))
            lsb = mctx.enter_context(tc.tile_pool(name="lsb", bufs=1))

            def xw(s, u):
                return x_all[:, u * HWP * B + s * SW * B:
                             u * HWP * B + (s + 1) * SW * B]

            # Software-pipelined emission: PE sees z MMs of slice s+2 and
            # ssum of s+1 between dependent ops of slice s, so it never
            # stalls in-order on the cross-engine softmax chain.
            state = {}

            def stage_z(s):
                # one 1.8MB wd DMA per slice, prefetched 2 slices ahead;
                # remaining x quarters ride between the first wd slices
                wd_sb = wdp.tile([128, 2 * SW * N], BF16, tag="wd",
                                 name=f"wd_{s}")
                nc.sync.dma_start(
                    wd_sb.rearrange("p (u t m) -> p u (t m)", u=2, t=SW),
                    wd_d[:, :, s * SW:(s + 1) * SW, :])
                z_ps = zp.tile([K, SLW], F32, tag="z", space="PSUM",
                               name=f"z_{s}")
                nc.tensor.matmul(z_ps, lhsT=cw_sb[:, 0:K], rhs=xw(s, 0),
                                 start=True, stop=False)
                nc.tensor.matmul(z_ps, lhsT=cw_sb[:, K:2 * K], rhs=xw(s, 1),
                                 start=False, stop=True)
                e_sb = ep.tile([K, SLW], F32, tag="e", name=f"e_{s}")
                nc.scalar.activation(e_sb, z_ps, AF.Exp)
                # bf16 exp copy: lets the softmax-sum matmul run at 1 cyc/row
                eb_sb = ep.tile([K, SLW], BF16, tag="eb", name=f"eb_{s}")
                nc.scalar.activation(eb_sb, z_ps, AF.Exp)
                state[s] = (wd_sb, e_sb, eb_sb)

            def stage_mid(s):
                wd_sb, e_sb, eb_sb = state[s]
                ssum_ps = sp.tile([K, SLW], F32, tag="ss", space="PSUM",
                                  name=f"ss_{s}")
                nc.tensor.matmul(ssum_ps, lhsT=ones8_sb, rhs=eb_sb,
                                 start=True, stop=True)
                r_sb = rp.tile([K, SLW], F32, tag="r", name=f"r_{s}")
                nc.vector.reciprocal_approx_fast(r_sb, ssum_ps)
                ahw = ah_full[:, s * SLW:(s + 1) * SLW]
                nc.vector.tensor_tensor(out=ahw, in0=e_sb, in1=r_sb,
                                        op=ALU.mult)
                # bf16 copy (on ACT) so the broadcast matmul is 1 cyc/row
                ah_bf = abf.tile([K, SLW], BF16, tag="ahbf", name=f"abf_{s}")
                nc.scalar.copy(ah_bf, ahw)
                state[s] = (wd_sb, ah_bf)

            def stage_heavy(s):
                wd_sb, ah_bf = state.pop(s)
                ahb_ps = abp.tile([128, SLW], F32, tag="ab", space="PSUM",
                                  name=f"ab_{s}")
                nc.tensor.matmul(ahb_ps, lhsT=selk_sb, rhs=ah_bf,
                                 start=True, stop=True)
                for nh in range(2):
                    y_sb = yp.tile([128, SLW], BF16, tag="y",
                                   name=f"y_{s}_{nh}")
                    nc.vector.tensor_tensor(out=y_sb, in0=xw(s, nh),
                                            in1=ahb_ps, op=ALU.mult)
                    for j in range(SW):
                        g = (s * 2 + nh) * SW + j     # global chunk ordinal
                        blk = g % 4                   # PE column group
                        nc.tensor.matmul(
                            dimred_ps[blk * B:(blk + 1) * B, :],
                            lhsT=y_sb[:, j * B:(j + 1) * B],
                            rhs=wd_sb[:, (nh * SW + j) * N:
                                      (nh * SW + j + 1) * N],
                            start=(blk > 0 and g == blk),
                            stop=(g >= 2 * NSL * SW - 4),
                            skip_group_check=True,
                            tile_position=(0, blk * B),
                        )


            def emit_loss():
                # ---- loss (overlaps the tail of the wd stream) ----
                # regather ah to [(k h), (w b)] via DRAM bounce on gpsimd
                ah_dram = dram.tile([K, H * W * B], F32, space="DRAM")
                nc.gpsimd.dma_start(ah_dram, ah_full[:, 0:H * W * B])
                A2 = lsb.tile([K * H, W * B], F32)
                nc.gpsimd.dma_start(
                    A2, ah_dram.rearrange("k (h rest) -> (k h) rest", h=H)
                )
                A2v = A2.rearrange("p (w b) -> p w b", b=B)
                G_ps = lp.tile([W, B * W], F32, tag="G", space="PSUM")
                for b in range(B):
                    ab = A2v[:, :, b:b + 1]
                    nc.tensor.matmul(G_ps[:, b * W:(b + 1) * W], lhsT=ab,
                                     rhs=ab, start=True, stop=True,
                                     skip_group_check=True)
                S_ps = lp.tile([K, W * B], F32, tag="S", space="PSUM")
                nc.tensor.matmul(S_ps, lhsT=hsel_sb, rhs=A2,
                                 start=True, stop=True)
                Gsq = lsb.tile([W, B * W], F32)
                nc.scalar.activation(Gsq, G_ps, AF.Square)
                Ssq = lsb.tile([K, W * B], F32)
                nc.scalar.activation(Ssq, S_ps, AF.Square)
                Gred = lsb.tile([W, B], F32)
                nc.vector.tensor_reduce(
                    Gred, Gsq.rearrange("p (b v) -> p b v", b=B),
                    axis=AX.X, op=ALU.add,
                )
                Sred = lsb.tile([K, B], F32)
                nc.vector.tensor_reduce(
                    Sred, Ssq.rearrange("p (w b) -> p b w", b=B),
                    axis=AX.X, op=ALU.add,
                )
                l_ps = lp.tile([1, B], F32, tag="l", space="PSUM")
                nc.tensor.matmul(l_ps, lhsT=ones14_sb, rhs=Gred,
                                 start=True, stop=False)
                nc.tensor.matmul(l_ps, lhsT=mones8_sb, rhs=Sred,
                                 start=False, stop=True)
                loss_sb = lsb.tile([1, B], F32)
                nc.vector.tensor_copy(loss_sb, l_ps)
                nc.gpsimd.dma_start(loss_d, loss_sb)

            LEAD = 2
            for s in range(NSL + LEAD):
                if s < NSL:
                    stage_z(s)
                if 1 <= s and s - 1 < NSL:
                    stage_mid(s - 1)
                if s == NSL:
                    emit_loss()
                if s >= LEAD:
                    stage_heavy(s - LEAD)

        # ---------------- tail ----------------
        with ExitStack() as tctx:
            tp = tctx.enter_context(tc.tile_pool(name="tail_sb", bufs=1))
            tpp = tctx.enter_context(
                tc.tile_pool(name="tail_ps", bufs=1, space="PSUM")
            )
            hp = tctx.enter_context(
                tc.tile_pool(name="hyp_ps", bufs=3, space="PSUM")
            )
            hs = tctx.enter_context(tc.tile_pool(name="hyp_sb", bufs=2))

            # merge + transpose fused: drT[n, b] = sum_g drg[32g+b, n]
            # = (drg^T @ sel4), two 128-col matmuls straight to [n, b].
            drg_sb = tp.tile([128, N], F32)
            nc.vector.tensor_copy(drg_sb, dimred_ps)
            drT_sb = tp.tile([128, 2 * B], F32)
            for nh in range(2):
                drT_ps = tpp.tile([128, B], F32, tag="drT", space="PSUM")
                nc.tensor.matmul(drT_ps, lhsT=drg_sb[:, nh * 128:(nh + 1) * 128],
                                 rhs=sel4_sb, start=True, stop=True)
                nc.vector.tensor_copy(drT_sb[:, nh * B:(nh + 1) * B], drT_ps)

            # hyp
            for c in range(CP // 128):
                hyp_ps = hp.tile([128, B], F32, tag="hyp", space="PSUM")
                nc.tensor.matmul(
                    hyp_ps, lhsT=wo_sb[:, c * 128:(c + 1) * 128],
                    rhs=drT_sb[:, 0:B], start=True, stop=False,
                )
                nc.tensor.matmul(
                    hyp_ps, lhsT=wo_sb[:, CP + c * 128:CP + (c + 1) * 128],
                    rhs=drT_sb[:, B:2 * B], start=False, stop=True,
                )
                hyp_sb = hs.tile([128, B], F32, tag="hyps")
                nc.scalar.activation(hyp_sb, hyp_ps, AF.Identity,
                                     bias=wob_sb[:, c:c + 1])
                nc.sync.dma_start(hyp_d[c], hyp_sb)

            # conf
            conf_ps = tpp.tile([1, B], F32, tag="conf", space="PSUM")
            nc.tensor.matmul(conf_ps, lhsT=wg_sb[:, 0:1], rhs=drT_sb[:, 0:B],
                             start=True, stop=False)
            nc.tensor.matmul(conf_ps, lhsT=wg_sb[:, 1:2], rhs=drT_sb[:, B:2 * B],
                             start=False, stop=True)
            conf_sb = tp.tile([1, B], F32)
            nc.scalar.activation(conf_sb, conf_ps, AF.Tanh, bias=wgb_sb[:, 0:1])
            nc.sync.dma_start(conf_d, conf_sb)


    return nc


def _bf16(a):
    return np.ascontiguousarray(a.astype(ml_dtypes.bfloat16))


def build_host_inputs(x, conv_w, dimred_w, dimred_b, Wo_w, Wo_b, Wg_w, Wg_b):
    """Returns in_maps: one dict per core."""
    x = np.asarray(x, np.float32)
    conv_w = np.asarray(conv_w, np.float32)
    dimred_w = np.asarray(dimred_w, np.float32)
    dimred_b = np.asarray(dimred_b, np.float32)
    Wo_w = np.asarray(Wo_w, np.float32)
    Wo_b = np.asarray(Wo_b, np.float32)
    Wg_w = np.asarray(Wg_w, np.float32)
    Wg_b = np.asarray(Wg_b, np.float32)

    # x_t[nh, nl, hw, b] = x[b, nh*128+nl, hw]  (partition-major contiguous)
    xt = x.transpose(1, 2, 3, 0).reshape(N, HW, B)          # [n, hw, b]
    xt = _bf16(xt.reshape(2, 128, HWP, B))

    # conv_w^T [2, 128, K]
    cwt = _bf16(conv_w.T.reshape(2, 128, K))

    shared = {
        "x_t": xt,
        "cw_t": cwt,
        "ones8": _bf16(np.ones((8, 8), np.float32)),
        "hsel": np.repeat(np.eye(K, dtype=np.float32), H, axis=0),
        "ones14": np.ones((14, 1), np.float32),
        "mones8": -np.ones((8, 1), np.float32),
        "onesb": np.ones((1, B), np.float32),
        "sel4": np.tile(np.eye(B, dtype=np.float32), (4, 1)),
    }

    in_maps = []
    for k in range(NCORES):
        # wd_t[nl, nh, hw, m] = dimred_w[k, m, n, hw]  (partition-outermost)
        wd = dimred_w[k].reshape(N, N, HW).transpose(1, 2, 0)   # [n, hw, m]
        wd = wd.reshape(2, 128, HWP, N).transpose(1, 0, 2, 3)   # [128, 2, hw, m]
        wo = np.zeros((CP, N), np.float32)
        wo[:C] = Wo_w[k]
        wob = np.zeros((CP,), np.float32)
        wob[:C] = Wo_b[k]
        selk = np.zeros((8, 128), np.float32)
        selk[k] = 1.0
        m = dict(shared)
        m.update({
            "wd_t": _bf16(wd),
            "wo_t": np.ascontiguousarray(wo.T.reshape(2, 128, CP)),
            "wob_t": wob.reshape(CP // 128, 128),
            "wg_t": np.ascontiguousarray(Wg_w[k].reshape(2, 128, 1)),
            "wgb_t": np.full((1, 1), Wg_b[k], np.float32),
            "db_t": dimred_b[k].reshape(1, N),
            "selk": _bf16(selk),
        })
        in_maps.append(m)
    return in_maps


def assemble_outputs(results):
    hyp = np.stack(
        [r["out_hyp"].reshape(CP, B)[:C].T for r in results], axis=1
    )                                                   # [B, K, C]
    conf = np.stack([r["out_conf"][0] for r in results], axis=1)[..., None]
    loss = results[0]["out_loss"][0][:, None]           # [B, 1]
    return (
        np.ascontiguousarray(hyp, np.float32),
        np.ascontiguousarray(conf, np.float32),
        np.ascontiguousarray(loss, np.float32),
    )


_GRAPH_CACHE = {}


def get_graph():
    if "nc" not in _GRAPH_CACHE:
        nc = build_graph()
        nc.finalize()
        _GRAPH_CACHE["nc"] = nc
    return _GRAPH_CACHE["nc"]


def kernel(**inputs):
    nc = get_graph()
    in_maps = build_host_inputs(**inputs)
    res = run_bass_kernel_spmd(nc, in_maps, core_ids=list(range(NCORES)))
    return assemble_outputs(res.results)


# revision 67
# speedup vs baseline: 1.1853x; 1.0849x over previous
"""Trainium2 Bass kernel for nn_Attention_9844065042780.

Sharding: expert-parallel over the K=8 independent groups, one group per
NeuronCore (8 cores).  Each core receives the full activations x (reordered
host-side), the full conv_w (to compute the shared softmax attention maps
and the shared orthogonality loss), and only its own group's
dimred/Wo/Wg weights.  Outputs are disjoint per-core slices (hyp[:,k,:],
conf[:,k]) plus the (identical on every core) loss, gathered host-side.

Per-core math (k = this core's group):
  z[k', hw, b]   = sum_n conv_w[k', n] x[b, n, hw]            (PE)
  ah[k', hw, b]  = softmax_k'(z)                              (ACT exp, PE sum, DVE recip/mul)
  y[(hw,n), b]   = ah[k, hw, b] * x[b, n, hw]                 (PE row-broadcast + DVE mul)
  dim_red[b, m]  = sum_{hw,n} y * wd[m, n, hw] + db[m]        (PE, 417 accumulating matmuls)
  hyp[c, b]      = sum_n Wo[c, n] dim_red[b, n] + Wo_b[c]     (PE + ACT bias)
  conf[b]        = tanh(sum_n Wg[n] dim_red[b, n] + Wg_b)     (PE + ACT)
  loss[b]        = ||A_b^T A_b||_F^2 - sum((H^T A_b)^2),  A_b = ah[:, :, :, b]  (PE/ACT/DVE)

Layouts (host-prepared, hw padded 196->208 so slices are uniform 16*32=512):
  x_t  [2, 208, 128, 32]  bf16   x_t[nh, hw, nl, b] = x[b, nh*128+nl, hw]
  wd_t [2, 208, 128, 256] bf16   wd_t[nh, hw, nl, m] = dimred_w[k, m, nh*128+nl, hw]
"""

import os
import numpy as np
import ml_dtypes
from contextlib import ExitStack

from concourse import bass, bacc, tile, mybir
from concourse.bass_utils import run_bass_kernel_spmd

F32 = mybir.dt.float32
BF16 = mybir.dt.bfloat16
AF = mybir.ActivationFunctionType
ALU = mybir.AluOpType
AX = mybir.AxisListType

B, N, H, W, K, C = 32, 256, 14, 14, 8, 1000
HW, HWP, CP = 196, 196, 1024
SW = 14                      # hw positions per slice (one h row)
NSL = HWP // SW              # 14 slices
SLW = SW * B                 # 448 free columns per slice
NCORES = 8


def build_graph():
    nc = bacc.Bacc("TRN2", target_bir_lowering=False, debug=False)

    def inp(name, shape, dtype):
        return nc.dram_tensor(name, shape, dtype, kind="ExternalInput").ap()

    def outp(name, shape, dtype):
        return nc.dram_tensor(name, shape, dtype, kind="ExternalOutput").ap()

    x_d = inp("x_t", [2, 128, HWP, B], BF16)
    wd_d = inp("wd_t", [128, 2, HWP, N], BF16)
    cw_d = inp("cw_t", [2, 128, K], BF16)
    wo_d = inp("wo_t", [2, 128, CP], F32)
    wob_d = inp("wob_t", [CP // 128, 128], F32)
    wg_d = inp("wg_t", [2, 128, 1], F32)
    wgb_d = inp("wgb_t", [1, 1], F32)
    db_d = inp("db_t", [1, N], F32)
    ones8_d = inp("ones8", [8, 8], BF16)
    selk_d = inp("selk", [8, 128], BF16)
    hsel_d = inp("hsel", [K * H, K], F32)
    ones14_d = inp("ones14", [14, 1], F32)
    mones8_d = inp("mones8", [8, 1], F32)
    onesb_d = inp("onesb", [1, B], F32)
    sel4_d = inp("sel4", [128, B], F32)

    hyp_d = outp("out_hyp", [CP // 128, 128, B], F32)
    conf_d = outp("out_conf", [1, B], F32)
    loss_d = outp("out_loss", [1, B], F32)

    with tile.TileContext(nc) as tc, ExitStack() as ctx:
        const = ctx.enter_context(tc.tile_pool(name="const", bufs=1))
        persist = ctx.enter_context(tc.tile_pool(name="persist", bufs=1))
        dram = ctx.enter_context(tc.tile_pool(name="dram", bufs=1, space="DRAM"))
        dr_pool = ctx.enter_context(
            tc.tile_pool(name="dr_psum", bufs=1, space="PSUM")
        )

        # PE warm-up first in program order: memset is gpsimd's first
        # instruction and the matmuls open the HAM clock gate
        # (1.2 -> 2.4 GHz) while the DMA pipeline ramps.
        with ExitStack() as wctx:
            wup = wctx.enter_context(tc.tile_pool(name="wup", bufs=1))
            wupp = wctx.enter_context(
                tc.tile_pool(name="wupp", bufs=1, space="PSUM"))
            wu_sb = wup.tile([128, 512], BF16)
            nc.gpsimd.memset(wu_sb, 0.0)
            wu_ps = wupp.tile([128, 512], F32, space="PSUM")
            for _ in range(12):
                nc.tensor.matmul(wu_ps, lhsT=wu_sb[:, 0:128], rhs=wu_sb,
                                 start=True, stop=True)

        # ---- constants into SBUF ----
        # Main-loop-critical consts go FIRST on the sync HWDGE ring (tiny);
        # tail-only consts go on the gpsimd SWDGE ring so neither the sync
        # ring (wd stream) nor the scalar engine (exp/copies) is blocked.
        cw_sb = const.tile([128, 2 * K], BF16)
        nc.sync.dma_start(cw_sb.rearrange("p (t c) -> p t c", t=2),
                          cw_d.rearrange("t p c -> p t c"))
        db_sb = const.tile([1, N], F32)
        nc.sync.dma_start(db_sb, db_d)
        ones8_sb = const.tile([8, 8], BF16)
        nc.sync.dma_start(ones8_sb, ones8_d)
        selk_sb = const.tile([8, 128], BF16)
        nc.sync.dma_start(selk_sb, selk_d)
        onesb_sb = const.tile([1, B], F32)
        nc.sync.dma_start(onesb_sb, onesb_d)

        # full attention map, all K groups: [8, (hw, b)] fp32
        ah_full = persist.tile([K, HWP * B], F32)

        # x resident in SBUF: [128, (u, hw, b)].  Loaded on the sync ring in
        # quarter-DMAs interleaved with the first wd slices (emitted in
        # stage_z below), first quarter up-front so slice 0 unblocks early.
        x_all = persist.tile([128, 2 * HWP * B], BF16)
        XH = HWP * B
        QHW = 49  # hw positions per x quarter-load (49*4 = 196)

        def load_x_quarter(q):
            lo, hi = q * QHW, (q + 1) * QHW
            for u in range(2):
                nc.sync.dma_start(
                    x_all[:, u * XH + lo * B:u * XH + hi * B],
                    x_d[u, :, lo:hi, :])

        load_x_quarter(0)

        wo_sb = const.tile([128, 2 * CP], F32)
        nc.gpsimd.dma_start(wo_sb.rearrange("p (t c) -> p t c", t=2),
                            wo_d.rearrange("t p c -> p t c"))
        wob_sb = const.tile([128, CP // 128], F32)
        nc.gpsimd.dma_start(wob_sb, wob_d.rearrange("c p -> p c"))
        wg_sb = const.tile([128, 2], F32)
        nc.gpsimd.dma_start(wg_sb.rearrange("p (t one) -> p t one", t=2),
                            wg_d.rearrange("t p one -> p t one"))
        wgb_sb = const.tile([1, 1], F32)
        nc.gpsimd.dma_start(wgb_sb, wgb_d)
        hsel_sb = const.tile([K * H, K], F32)
        nc.gpsimd.dma_start(hsel_sb, hsel_d)
        ones14_sb = const.tile([14, 1], F32)
        nc.gpsimd.dma_start(ones14_sb, ones14_d)
        mones8_sb = const.tile([8, 1], F32)
        nc.gpsimd.dma_start(mones8_sb, mones8_d)
        sel4_sb = const.tile([128, B], F32)
        nc.gpsimd.dma_start(sel4_sb, sel4_d)

        # dim_red accumulator: four 32-partition blocks (one per PE column
        # group) accumulate concurrently; merged after the main loop.
        dimred_ps = dr_pool.tile([128, N], F32, space="PSUM")
        # bias seeding matmul into block 0: dimred[b, m] = 1 * db[m]
        nc.tensor.matmul(
            dimred_ps[0:B, :], lhsT=onesb_sb, rhs=db_sb, start=True,
            stop=False, skip_group_check=True, tile_position=(0, 0),
        )

        with ExitStack() as mctx:
            wdp = mctx.enter_context(tc.tile_pool(name="wdp", bufs=6))
            ep = mctx.enter_context(tc.tile_pool(name="ep", bufs=4))
            abf = mctx.enter_context(tc.tile_pool(name="abf", bufs=3))
            rp = mctx.enter_context(tc.tile_pool(name="rp", bufs=3))
            yp = mctx.enter_context(tc.tile_pool(name="yp", bufs=4))
            zp = mctx.enter_context(tc.tile_pool(name="zp", bufs=2, space="PSUM"))
            sp = mctx.enter_context(tc.tile_pool(name="sp", bufs=1, space="PSUM"))
            abp = mctx.enter_context(tc.tile_pool(name="abp", bufs=1, space="PSUM"))
            lp = mctx.enter_context(tc.tile_pool(name="lp", bufs=1, space="PSUM"))
            lsb = mctx.enter_context(tc.tile_pool(name="lsb", bufs=1))

            def xw(s, u):
                return x_all[:, u * HWP * B + s * SW * B:
                             u * HWP * B + (s + 1) * SW * B]

            # Software-pipelined emission: PE sees z MMs of slice s+2 and
            # ssum of s+1 between dependent ops of slice s, so it never
            # stalls in-order on the cross-engine softmax chain.
            state = {}

            def stage_z(s):
                # one 1.8MB wd DMA per slice, prefetched 2 slices ahead;
                # remaining x quarters ride between the first wd slices
                wd_sb = wdp.tile([128, 2 * SW * N], BF16, tag="wd",
                                 name=f"wd_{s}")
                nc.sync.dma_start(
                    wd_sb.rearrange("p (u t m) -> p u (t m)", u=2, t=SW),
                    wd_d[:, :, s * SW:(s + 1) * SW, :])
                if 1 <= s <= 3:
                    load_x_quarter(s)
                z_ps = zp.tile([K, SLW], F32, tag="z", space="PSUM",
                               name=f"z_{s}")
                nc.tensor.matmul(z_ps, lhsT=cw_sb[:, 0:K], rhs=xw(s, 0),
                                 start=True, stop=False)
                nc.tensor.matmul(z_ps, lhsT=cw_sb[:, K:2 * K], rhs=xw(s, 1),
                                 start=False, stop=True)
                e_sb = ep.tile([K, SLW], F32, tag="e", name=f"e_{s}")
                nc.scalar.activation(e_sb, z_ps, AF.Exp)
                # bf16 exp copy: lets the softmax-sum matmul run at 1 cyc/row
                eb_sb = ep.tile([K, SLW], BF16, tag="eb", name=f"eb_{s}")
                nc.scalar.activation(eb_sb, z_ps, AF.Exp)
                state[s] = (wd_sb, e_sb, eb_sb)

            def stage_mid(s):
                wd_sb, e_sb, eb_sb = state[s]
                ssum_ps = sp.tile([K, SLW], F32, tag="ss", space="PSUM",
                                  name=f"ss_{s}")
                nc.tensor.matmul(ssum_ps, lhsT=ones8_sb, rhs=eb_sb,
                                 start=True, stop=True)
                r_sb = rp.tile([K, SLW], F32, tag="r", name=f"r_{s}")
                nc.vector.reciprocal_approx_fast(r_sb, ssum_ps)
                ahw = ah_full[:, s * SLW:(s + 1) * SLW]
                nc.vector.tensor_tensor(out=ahw, in0=e_sb, in1=r_sb,
                                        op=ALU.mult)
                # bf16 copy (on ACT) so the broadcast matmul is 1 cyc/row
                ah_bf = abf.tile([K, SLW], BF16, tag="ahbf", name=f"abf_{s}")
                nc.scalar.copy(ah_bf, ahw)
                state[s] = (wd_sb, ah_bf)

            def stage_heavy(s):
                wd_sb, ah_bf = state.pop(s)
                ahb_ps = abp.tile([128, SLW], F32, tag="ab", space="PSUM",
                                  name=f"ab_{s}")
                nc.tensor.matmul(ahb_ps, lhsT=selk_sb, rhs=ah_bf,
                                 start=True, stop=True)
                for nh in range(2):
                    y_sb = yp.tile([128, SLW], BF16, tag="y",
                                   name=f"y_{s}_{nh}")
                    nc.vector.tensor_tensor(out=y_sb, in0=xw(s, nh),
                                            in1=ahb_ps, op=ALU.mult)
                    for j in range(SW):
                        g = (s * 2 + nh) * SW + j     # global chunk ordinal
                        blk = g % 4                   # PE column group
                        nc.tensor.matmul(
                            dimred_ps[blk * B:(blk + 1) * B, :],
                            lhsT=y_sb[:, j * B:(j + 1) * B],
                            rhs=wd_sb[:, (nh * SW + j) * N:
                                      (nh * SW + j + 1) * N],
                            start=(blk > 0 and g == blk),
                            stop=(g >= 2 * NSL * SW - 4),
                            skip_group_check=True,
                            tile_position=(0, blk * B),
                        )


            def emit_loss():
                # ---- loss (overlaps the tail of the wd stream) ----
                # regather ah to [(k h), (w b)] via DRAM bounce on gpsimd
                ah_dram = dram.tile([K, H * W * B], F32, space="DRAM")
                nc.gpsimd.dma_start(ah_dram, ah_full[:, 0:H * W * B])
                A2 = lsb.tile([K * H, W * B], F32)
                nc.gpsimd.dma_start(
                    A2, ah_dram.rearrange("k (h rest) -> (k h) rest", h=H)
                )
                A2v = A2.rearrange("p (w b) -> p w b", b=B)
                G_ps = lp.tile([W, B * W], F32, tag="G", space="PSUM")
                for b in range(B):
                    ab = A2v[:, :, b:b + 1]
                    nc.tensor.matmul(G_ps[:, b * W:(b + 1) * W], lhsT=ab,
                                     rhs=ab, start=True, stop=True,
                                     skip_group_check=True)
                S_ps = lp.tile([K, W * B], F32, tag="S", space="PSUM")
                nc.tensor.matmul(S_ps, lhsT=hsel_sb, rhs=A2,
                                 start=True, stop=True)
                Gsq = lsb.tile([W, B * W], F32)
                nc.scalar.activation(Gsq, G_ps, AF.Square)
                Ssq = lsb.tile([K, W * B], F32)
                nc.scalar.activation(Ssq, S_ps, AF.Square)
                Gred = lsb.tile([W, B], F32)
                nc.vector.tensor_reduce(
                    Gred, Gsq.rearrange("p (b v) -> p b v", b=B),
                    axis=AX.X, op=ALU.add,
                )
                Sred = lsb.tile([K, B], F32)
                nc.vector.tensor_reduce(
                    Sred, Ssq.rearrange("p (w b) -> p b w", b=B),
                    axis=AX.X, op=ALU.add,
                )
                l_ps = lp.tile([1, B], F32, tag="l", space="PSUM")
                nc.tensor.matmul(l_ps, lhsT=ones14_sb, rhs=Gred,
                                 start=True, stop=False)
                nc.tensor.matmul(l_ps, lhsT=mones8_sb, rhs=Sred,
                                 start=False, stop=True)
                loss_sb = lsb.tile([1, B], F32)
                nc.vector.tensor_copy(loss_sb, l_ps)
                nc.gpsimd.dma_start(loss_d, loss_sb)

            LEAD = 2
            for s in range(NSL + LEAD):
                if s < NSL:
                    stage_z(s)
                if 1 <= s and s - 1 < NSL:
                    stage_mid(s - 1)
                if s == NSL:
                    emit_loss()
                if s >= LEAD:
                    stage_heavy(s - LEAD)

        # ---------------- tail ----------------
        with ExitStack() as tctx:
            tp = tctx.enter_context(tc.tile_pool(name="tail_sb", bufs=1))
            tpp = tctx.enter_context(
                tc.tile_pool(name="tail_ps", bufs=1, space="PSUM")
            )
            hp = tctx.enter_context(
                tc.tile_pool(name="hyp_ps", bufs=3, space="PSUM")
            )
            hs = tctx.enter_context(tc.tile_pool(name="hyp_sb", bufs=2))

            # merge + transpose fused: drT[n, b] = sum_g drg[32g+b, n]
            # = (drg^T @ sel4), two 128-col matmuls straight to [n, b].
            drg_sb = tp.tile([128, N], F32)
            nc.vector.tensor_copy(drg_sb, dimred_ps)
            drT_sb = tp.tile([128, 2 * B], F32)
            for nh in range(2):
                drT_ps = tpp.tile([128, B], F32, tag="drT", space="PSUM")
                nc.tensor.matmul(drT_ps, lhsT=drg_sb[:, nh * 128:(nh + 1) * 128],
                                 rhs=sel4_sb, start=True, stop=True)
                nc.vector.tensor_copy(drT_sb[:, nh * B:(nh + 1) * B], drT_ps)

            # hyp
            for c in range(CP // 128):
                hyp_ps = hp.tile([128, B], F32, tag="hyp", space="PSUM")
                nc.tensor.matmul(
                    hyp_ps, lhsT=wo_sb[:, c * 128:(c + 1) * 128],
                    rhs=drT_sb[:, 0:B], start=True, stop=False,
                )
                nc.tensor.matmul(
                    hyp_ps, lhsT=wo_sb[:, CP + c * 128:CP + (c + 1) * 128],
                    rhs=drT_sb[:, B:2 * B], start=False, stop=True,
                )
                hyp_sb = hs.tile([128, B], F32, tag="hyps")
                nc.scalar.activation(hyp_sb, hyp_ps, AF.Identity,
                                     bias=wob_sb[:, c:c + 1])
                nc.sync.dma_start(hyp_d[c], hyp_sb)

            # conf
            conf_ps = tpp.tile([1, B], F32, tag="conf", space="PSUM")
            nc.tensor.matmul(conf_ps, lhsT=wg_sb[:, 0:1], rhs=drT_sb[:, 0:B],
                             start=True, stop=False)
            nc.tensor.matmul(conf_ps, lhsT=wg_sb[:, 1:2], rhs=drT_sb[:, B:2 * B],
                             start=False, stop=True)
            conf_sb = tp.tile([1, B], F32)
            nc.scalar.activation(conf_sb, conf_ps, AF.Tanh, bias=wgb_sb[:, 0:1])
            nc.sync.dma_start(conf_d, conf_sb)


    return nc


def _bf16(a):
    return np.ascontiguousarray(a.astype(ml_dtypes.bfloat16))


def build_host_inputs(x, conv_w, dimred_w, dimred_b, Wo_w, Wo_b, Wg_w, Wg_b):
    """Returns in_maps: one dict per core."""
    x = np.asarray(x, np.float32)
    conv_w = np.asarray(conv_w, np.float32)
    dimred_w = np.asarray(dimred_w, np.float32)
    dimred_b = np.asarray(dimred_b, np.float32)
    Wo_w = np.asarray(Wo_w, np.float32)
    Wo_b = np.asarray(Wo_b, np.float32)
    Wg_w = np.asarray(Wg_w, np.float32)
    Wg_b = np.asarray(Wg_b, np.float32)

    # x_t[nh, nl, hw, b] = x[b, nh*128+nl, hw]  (partition-major contiguous)
    xt = x.transpose(1, 2, 3, 0).reshape(N, HW, B)          # [n, hw, b]
    xt = _bf16(xt.reshape(2, 128, HWP, B))

    # conv_w^T [2, 128, K]
    cwt = _bf16(conv_w.T.reshape(2, 128, K))

    shared = {
        "x_t": xt,
        "cw_t": cwt,
        "ones8": _bf16(np.ones((8, 8), np.float32)),
        "hsel": np.repeat(np.eye(K, dtype=np.float32), H, axis=0),
        "ones14": np.ones((14, 1), np.float32),
        "mones8": -np.ones((8, 1), np.float32),
        "onesb": np.ones((1, B), np.float32),
        "sel4": np.tile(np.eye(B, dtype=np.float32), (4, 1)),
    }

    in_maps = []
    for k in range(NCORES):
        # wd_t[nl, nh, hw, m] = dimred_w[k, m, n, hw]  (partition-outermost)
        wd = dimred_w[k].reshape(N, N, HW).transpose(1, 2, 0)   # [n, hw, m]
        wd = wd.reshape(2, 128, HWP, N).transpose(1, 0, 2, 3)   # [128, 2, hw, m]
        wo = np.zeros((CP, N), np.float32)
        wo[:C] = Wo_w[k]
        wob = np.zeros((CP,), np.float32)
        wob[:C] = Wo_b[k]
        selk = np.zeros((8, 128), np.float32)
        selk[k] = 1.0
        m = dict(shared)
        m.update({
            "wd_t": _bf16(wd),
            "wo_t": np.ascontiguousarray(wo.T.reshape(2, 128, CP)),
            "wob_t": wob.reshape(CP // 128, 128),
            "wg_t": np.ascontiguousarray(Wg_w[k].reshape(2, 128, 1)),
            "wgb_t": np.full((1, 1), Wg_b[k], np.float32),
            "db_t": dimred_b[k].reshape(1, N),
            "selk": _bf16(selk),
        })
        in_maps.append(m)
    return in_maps


def assemble_outputs(results):
    hyp = np.stack(
        [r["out_hyp"].reshape(CP, B)[:C].T for r in results], axis=1
    )                                                   # [B, K, C]
    conf = np.stack([r["out_conf"][0] for r in results], axis=1)[..., None]
    loss = results[0]["out_loss"][0][:, None]           # [B, 1]
    return (
        np.ascontiguousarray(hyp, np.float32),
        np.ascontiguousarray(conf, np.float32),
        np.ascontiguousarray(loss, np.float32),
    )


_GRAPH_CACHE = {}


def get_graph():
    if "nc" not in _GRAPH_CACHE:
        nc = build_graph()
        nc.finalize()
        _GRAPH_CACHE["nc"] = nc
    return _GRAPH_CACHE["nc"]


def kernel(**inputs):
    nc = get_graph()
    in_maps = build_host_inputs(**inputs)
    res = run_bass_kernel_spmd(nc, in_maps, core_ids=list(range(NCORES)))
    return assemble_outputs(res.results)


# revision 69
# speedup vs baseline: 1.2050x; 1.0167x over previous
"""Trainium2 Bass kernel for nn_Attention_9844065042780.

Sharding: expert-parallel over the K=8 independent groups, one group per
NeuronCore (8 cores).  Each core receives the full activations x (reordered
host-side), the full conv_w (to compute the shared softmax attention maps
and the shared orthogonality loss), and only its own group's
dimred/Wo/Wg weights.  Outputs are disjoint per-core slices (hyp[:,k,:],
conf[:,k]) plus the (identical on every core) loss, gathered host-side.

Per-core math (k = this core's group):
  z[k', hw, b]   = sum_n conv_w[k', n] x[b, n, hw]            (PE)
  ah[k', hw, b]  = softmax_k'(z)                              (ACT exp, PE sum, DVE recip/mul)
  y[(hw,n), b]   = ah[k, hw, b] * x[b, n, hw]                 (PE row-broadcast + DVE mul)
  dim_red[b, m]  = sum_{hw,n} y * wd[m, n, hw] + db[m]        (PE, 393 accumulating matmuls
                   packed 4-wide across the PE column groups via tile_position)
  hyp[c, b]      = sum_n Wo[c, n] dim_red[b, n] + Wo_b[c]     (PE + ACT bias)
  conf[b]        = tanh(sum_n Wg[n] dim_red[b, n] + Wg_b)     (PE + ACT)
  loss[b]        = ||A_b^T A_b||_F^2 - sum((H^T A_b)^2),  A_b = ah[:, :, :, b]  (PE/ACT/DVE)

Pipeline: 14 hw-slices (one h row each, 448 free columns).  Emission is
software-pipelined (z two slices ahead, softmax-mid one ahead, the heavy
wd-matmul stage behind) so the PE never stalls in-order on the cross-engine
softmax chain; wd streams on the sync HWDGE ring at ~2MB per slice with the
x quarters interleaved; the loss block is emitted before the last two heavy
stages so it overlaps the final wd waits.

Layouts (host-prepared, partition-major so every DMA is contiguous
per partition):
  x_t  [2, 128, 196, 32]  bf16   x_t[nh, nl, hw, b] = x[b, nh*128+nl, hw]
  wd_t [128, 2, 196, 256] bf16   wd_t[nl, nh, hw, m] = dimred_w[k, m, nh*128+nl, hw]
"""

import os
import numpy as np
import ml_dtypes
from contextlib import ExitStack

from concourse import bass, bacc, tile, mybir
from concourse.bass_utils import run_bass_kernel_spmd

F32 = mybir.dt.float32
BF16 = mybir.dt.bfloat16
AF = mybir.ActivationFunctionType
ALU = mybir.AluOpType
AX = mybir.AxisListType

B, N, H, W, K, C = 32, 256, 14, 14, 8, 1000
HW, HWP, CP = 196, 196, 1024
SW = 14                      # hw positions per slice (one h row)
NSL = HWP // SW              # 14 slices
SLW = SW * B                 # 448 free columns per slice
NCORES = 8


def build_graph():
    nc = bacc.Bacc("TRN2", target_bir_lowering=False, debug=False)

    def inp(name, shape, dtype):
        return nc.dram_tensor(name, shape, dtype, kind="ExternalInput").ap()

    def outp(name, shape, dtype):
        return nc.dram_tensor(name, shape, dtype, kind="ExternalOutput").ap()

    x_d = inp("x_t", [2, 128, HWP, B], BF16)
    wd_d = inp("wd_t", [128, 2, HWP, N], BF16)
    cw_d = inp("cw_t", [2, 128, K], BF16)
    wo_d = inp("wo_t", [2, 128, CP], BF16)
    wob_d = inp("wob_t", [CP // 128, 128], F32)
    wg_d = inp("wg_t", [2, 128, 1], F32)
    wgb_d = inp("wgb_t", [1, 1], F32)
    db_d = inp("db_t", [1, N], F32)
    ones8_d = inp("ones8", [8, 8], BF16)
    selk_d = inp("selk", [8, 128], BF16)
    hsel_d = inp("hsel", [K * H, K], F32)
    ones14_d = inp("ones14", [14, 1], F32)
    mones8_d = inp("mones8", [8, 1], F32)
    onesb_d = inp("onesb", [1, B], F32)
    sel4_d = inp("sel4", [128, B], F32)

    hyp_d = outp("out_hyp", [CP // 128, 128, B], F32)
    conf_d = outp("out_conf", [1, B], F32)
    loss_d = outp("out_loss", [1, B], F32)

    with tile.TileContext(nc) as tc, ExitStack() as ctx:
        const = ctx.enter_context(tc.tile_pool(name="const", bufs=1))
        persist = ctx.enter_context(tc.tile_pool(name="persist", bufs=1))
        dram = ctx.enter_context(tc.tile_pool(name="dram", bufs=1, space="DRAM"))
        dr_pool = ctx.enter_context(
            tc.tile_pool(name="dr_psum", bufs=1, space="PSUM")
        )

        # PE warm-up first in program order: memset is gpsimd's first
        # instruction and the matmuls open the HAM clock gate
        # (1.2 -> 2.4 GHz) while the DMA pipeline ramps.
        with ExitStack() as wctx:
            wup = wctx.enter_context(tc.tile_pool(name="wup", bufs=1))
            wupp = wctx.enter_context(
                tc.tile_pool(name="wupp", bufs=1, space="PSUM"))
            wu_sb = wup.tile([128, 512], BF16)
            nc.gpsimd.memset(wu_sb, 0.0)
            wu_ps = wupp.tile([128, 512], F32, space="PSUM")
            for _ in range(12):
                nc.tensor.matmul(wu_ps, lhsT=wu_sb[:, 0:128], rhs=wu_sb,
                                 start=True, stop=True)

        # ---- constants into SBUF ----
        # Main-loop-critical consts go FIRST on the sync HWDGE ring (tiny);
        # tail-only consts go on the gpsimd SWDGE ring so neither the sync
        # ring (wd stream) nor the scalar engine (exp/copies) is blocked.
        cw_sb = const.tile([128, 2 * K], BF16)
        nc.sync.dma_start(cw_sb.rearrange("p (t c) -> p t c", t=2),
                          cw_d.rearrange("t p c -> p t c"))
        db_sb = const.tile([1, N], F32)
        nc.sync.dma_start(db_sb, db_d)
        ones8_sb = const.tile([8, 8], BF16)
        nc.sync.dma_start(ones8_sb, ones8_d)
        selk_sb = const.tile([8, 128], BF16)
        nc.sync.dma_start(selk_sb, selk_d)
        onesb_sb = const.tile([1, B], F32)
        nc.sync.dma_start(onesb_sb, onesb_d)

        # full attention map, all K groups: [8, (hw, b)] fp32
        ah_full = persist.tile([K, HWP * B], F32)

        # x resident in SBUF: [128, (u, hw, b)].  Loaded on the sync ring in
        # quarter-DMAs interleaved with the first wd slices (emitted in
        # stage_z below), first quarter up-front so slice 0 unblocks early.
        x_all = persist.tile([128, 2 * HWP * B], BF16)
        XH = HWP * B
        QHW = 49  # hw positions per x quarter-load (49*4 = 196)

        def load_x_quarter(q):
            lo, hi = q * QHW, (q + 1) * QHW
            for u in range(2):
                nc.sync.dma_start(
                    x_all[:, u * XH + lo * B:u * XH + hi * B],
                    x_d[u, :, lo:hi, :])

        load_x_quarter(0)

        wo_sb = const.tile([128, 2 * CP], BF16)
        nc.gpsimd.dma_start(wo_sb.rearrange("p (t c) -> p t c", t=2),
                            wo_d.rearrange("t p c -> p t c"))
        wob_sb = const.tile([128, CP // 128], F32)
        nc.gpsimd.dma_start(wob_sb, wob_d.rearrange("c p -> p c"))
        wg_sb = const.tile([128, 2], F32)
        nc.gpsimd.dma_start(wg_sb.rearrange("p (t one) -> p t one", t=2),
                            wg_d.rearrange("t p one -> p t one"))
        wgb_sb = const.tile([1, 1], F32)
        nc.gpsimd.dma_start(wgb_sb, wgb_d)
        hsel_sb = const.tile([K * H, K], F32)
        nc.gpsimd.dma_start(hsel_sb, hsel_d)
        ones14_sb = const.tile([14, 1], F32)
        nc.gpsimd.dma_start(ones14_sb, ones14_d)
        mones8_sb = const.tile([8, 1], F32)
        nc.gpsimd.dma_start(mones8_sb, mones8_d)
        sel4_sb = const.tile([128, B], F32)
        nc.gpsimd.dma_start(sel4_sb, sel4_d)

        # dim_red accumulator: four 32-partition blocks (one per PE column
        # group) accumulate concurrently; merged after the main loop.
        dimred_ps = dr_pool.tile([128, N], F32, space="PSUM")
        # bias seeding matmul into block 0: dimred[b, m] = 1 * db[m]
        nc.tensor.matmul(
            dimred_ps[0:B, :], lhsT=onesb_sb, rhs=db_sb, start=True,
            stop=False, skip_group_check=True, tile_position=(0, 0),
        )

        with ExitStack() as mctx:
            wdp = mctx.enter_context(tc.tile_pool(name="wdp", bufs=6))
            ep = mctx.enter_context(tc.tile_pool(name="ep", bufs=4))
            abf = mctx.enter_context(tc.tile_pool(name="abf", bufs=3))
            rp = mctx.enter_context(tc.tile_pool(name="rp", bufs=3))
            yp = mctx.enter_context(tc.tile_pool(name="yp", bufs=4))
            zp = mctx.enter_context(tc.tile_pool(name="zp", bufs=2, space="PSUM"))
            sp = mctx.enter_context(tc.tile_pool(name="sp", bufs=1, space="PSUM"))
            abp = mctx.enter_context(tc.tile_pool(name="abp", bufs=1, space="PSUM"))
            lp = mctx.enter_context(tc.tile_pool(name="lp", bufs=1, space="PSUM"))
            lsb = mctx.enter_context(tc.tile_pool(name="lsb", bufs=1))

            def xw(s, u):
                return x_all[:, u * HWP * B + s * SW * B:
                             u * HWP * B + (s + 1) * SW * B]

            # Software-pipelined emission: PE sees z MMs of slice s+2 and
            # ssum of s+1 between dependent ops of slice s, so it never
            # stalls in-order on the cross-engine softmax chain.
            state = {}

            def stage_z(s):
                # one 1.8MB wd DMA per slice, prefetched 2 slices ahead;
                # remaining x quarters ride between the first wd slices
                wd_sb = wdp.tile([128, 2 * SW * N], BF16, tag="wd",
                                 name=f"wd_{s}")
                nc.sync.dma_start(
                    wd_sb.rearrange("p (u t m) -> p u (t m)", u=2, t=SW),
                    wd_d[:, :, s * SW:(s + 1) * SW, :])
                if 1 <= s <= 3:
                    load_x_quarter(s)
                z_ps = zp.tile([K, SLW], F32, tag="z", space="PSUM",
                               name=f"z_{s}")
                nc.tensor.matmul(z_ps, lhsT=cw_sb[:, 0:K], rhs=xw(s, 0),
                                 start=True, stop=False)
                nc.tensor.matmul(z_ps, lhsT=cw_sb[:, K:2 * K], rhs=xw(s, 1),
                                 start=False, stop=True)
                e_sb = ep.tile([K, SLW], F32, tag="e", name=f"e_{s}")
                nc.scalar.activation(e_sb, z_ps, AF.Exp)
                # bf16 exp copy: lets the softmax-sum matmul run at 1 cyc/row
                eb_sb = ep.tile([K, SLW], BF16, tag="eb", name=f"eb_{s}")
                nc.scalar.activation(eb_sb, z_ps, AF.Exp)
                state[s] = (wd_sb, e_sb, eb_sb)

            def stage_mid(s):
                wd_sb, e_sb, eb_sb = state[s]
                ssum_ps = sp.tile([K, SLW], F32, tag="ss", space="PSUM",
                                  name=f"ss_{s}")
                nc.tensor.matmul(ssum_ps, lhsT=ones8_sb, rhs=eb_sb,
                                 start=True, stop=True)
                r_sb = rp.tile([K, SLW], F32, tag="r", name=f"r_{s}")
                nc.vector.reciprocal_approx_fast(r_sb, ssum_ps)
                ahw = ah_full[:, s * SLW:(s + 1) * SLW]
                nc.vector.tensor_tensor(out=ahw, in0=e_sb, in1=r_sb,
                                        op=ALU.mult)
                # bf16 copy (on ACT) so the broadcast matmul is 1 cyc/row
                ah_bf = abf.tile([K, SLW], BF16, tag="ahbf", name=f"abf_{s}")
                nc.scalar.copy(ah_bf, ahw)
                state[s] = (wd_sb, ah_bf)

            def stage_heavy(s):
                wd_sb, ah_bf = state.pop(s)
                ahb_ps = abp.tile([128, SLW], F32, tag="ab", space="PSUM",
                                  name=f"ab_{s}")
                nc.tensor.matmul(ahb_ps, lhsT=selk_sb, rhs=ah_bf,
                                 start=True, stop=True)
                for nh in range(2):
                    y_sb = yp.tile([128, SLW], BF16, tag="y",
                                   name=f"y_{s}_{nh}")
                    nc.vector.tensor_tensor(out=y_sb, in0=xw(s, nh),
                                            in1=ahb_ps, op=ALU.mult)
                    for j in range(SW):
                        g = (s * 2 + nh) * SW + j     # global chunk ordinal
                        blk = g % 4                   # PE column group
                        nc.tensor.matmul(
                            dimred_ps[blk * B:(blk + 1) * B, :],
                            lhsT=y_sb[:, j * B:(j + 1) * B],
                            rhs=wd_sb[:, (nh * SW + j) * N:
                                      (nh * SW + j + 1) * N],
                            start=(blk > 0 and g == blk),
                            stop=(g >= 2 * NSL * SW - 4),
                            skip_group_check=True,
                            tile_position=(0, blk * B),
                        )


            def emit_loss():
                # ---- loss (overlaps the tail of the wd stream) ----
                # regather ah to [(k h), (w b)] via DRAM bounce on gpsimd
                ah_dram = dram.tile([K, H * W * B], F32, space="DRAM")
                nc.gpsimd.dma_start(ah_dram, ah_full[:, 0:H * W * B])
                A2 = lsb.tile([K * H, W * B], F32)
                nc.gpsimd.dma_start(
                    A2, ah_dram.rearrange("k (h rest) -> (k h) rest", h=H)
                )
                A2v = A2.rearrange("p (w b) -> p w b", b=B)
                G_ps = lp.tile([W, B * W], F32, tag="G", space="PSUM")
                for b in range(B):
                    ab = A2v[:, :, b:b + 1]
                    nc.tensor.matmul(G_ps[:, b * W:(b + 1) * W], lhsT=ab,
                                     rhs=ab, start=True, stop=True,
                                     skip_group_check=True)
                S_ps = lp.tile([K, W * B], F32, tag="S", space="PSUM")
                nc.tensor.matmul(S_ps, lhsT=hsel_sb, rhs=A2,
                                 start=True, stop=True)
                Gsq = lsb.tile([W, B * W], F32)
                nc.scalar.activation(Gsq, G_ps, AF.Square)
                Ssq = lsb.tile([K, W * B], F32)
                nc.scalar.activation(Ssq, S_ps, AF.Square)
                Gred = lsb.tile([W, B], F32)
                nc.vector.tensor_reduce(
                    Gred, Gsq.rearrange("p (b v) -> p b v", b=B),
                    axis=AX.X, op=ALU.add,
                )
                Sred = lsb.tile([K, B], F32)
                nc.vector.tensor_reduce(
                    Sred, Ssq.rearrange("p (w b) -> p b w", b=B),
                    axis=AX.X, op=ALU.add,
                )
                l_ps = lp.tile([1, B], F32, tag="l", space="PSUM")
                nc.tensor.matmul(l_ps, lhsT=ones14_sb, rhs=Gred,
                                 start=True, stop=False)
                nc.tensor.matmul(l_ps, lhsT=mones8_sb, rhs=Sred,
                                 start=False, stop=True)
                loss_sb = lsb.tile([1, B], F32)
                nc.vector.tensor_copy(loss_sb, l_ps)
                nc.gpsimd.dma_start(loss_d, loss_sb)

            LEAD = 2
            for s in range(NSL + LEAD):
                if s < NSL:
                    stage_z(s)
                if 1 <= s and s - 1 < NSL:
                    stage_mid(s - 1)
                if s == NSL:
                    emit_loss()
                if s >= LEAD:
                    stage_heavy(s - LEAD)

        # ---------------- tail ----------------
        with ExitStack() as tctx:
            tp = tctx.enter_context(tc.tile_pool(name="tail_sb", bufs=1))
            tpp = tctx.enter_context(
                tc.tile_pool(name="tail_ps", bufs=1, space="PSUM")
            )
            hp = tctx.enter_context(
                tc.tile_pool(name="hyp_ps", bufs=3, space="PSUM")
            )
            hs = tctx.enter_context(tc.tile_pool(name="hyp_sb", bufs=2))

            # merge + transpose fused: drT[n, b] = sum_g drg[32g+b, n]
            # = (drg^T @ sel4), two 128-col matmuls straight to [n, b].
            drg_sb = tp.tile([128, N], F32)
            nc.vector.tensor_copy(drg_sb, dimred_ps)
            drT_sb = tp.tile([128, 2 * B], F32)
            drT_bf = tp.tile([128, 2 * B], BF16)
            for nh in range(2):
                drT_ps = tpp.tile([128, B], F32, tag="drT", space="PSUM")
                nc.tensor.matmul(drT_ps, lhsT=drg_sb[:, nh * 128:(nh + 1) * 128],
                                 rhs=sel4_sb, start=True, stop=True)
                nc.vector.tensor_copy(drT_sb[:, nh * B:(nh + 1) * B], drT_ps)
                nc.scalar.copy(drT_bf[:, nh * B:(nh + 1) * B], drT_ps)

            # hyp
            for c in range(CP // 128):
                hyp_ps = hp.tile([128, B], F32, tag="hyp", space="PSUM")
                nc.tensor.matmul(
                    hyp_ps, lhsT=wo_sb[:, c * 128:(c + 1) * 128],
                    rhs=drT_bf[:, 0:B], start=True, stop=False,
                )
                nc.tensor.matmul(
                    hyp_ps, lhsT=wo_sb[:, CP + c * 128:CP + (c + 1) * 128],
                    rhs=drT_bf[:, B:2 * B], start=False, stop=True,
                )
                hyp_sb = hs.tile([128, B], F32, tag="hyps")
                nc.scalar.activation(hyp_sb, hyp_ps, AF.Identity,
                                     bias=wob_sb[:, c:c + 1])
                nc.sync.dma_start(hyp_d[c], hyp_sb)

            # conf
            conf_ps = tpp.tile([1, B], F32, tag="conf", space="PSUM")
            nc.tensor.matmul(conf_ps, lhsT=wg_sb[:, 0:1], rhs=drT_sb[:, 0:B],
                             start=True, stop=False)
            nc.tensor.matmul(conf_ps, lhsT=wg_sb[:, 1:2], rhs=drT_sb[:, B:2 * B],
                             start=False, stop=True)
            conf_sb = tp.tile([1, B], F32)
            nc.scalar.activation(conf_sb, conf_ps, AF.Tanh, bias=wgb_sb[:, 0:1])
            nc.sync.dma_start(conf_d, conf_sb)


    return nc


def _bf16(a):
    return np.ascontiguousarray(a.astype(ml_dtypes.bfloat16))


def build_host_inputs(x, conv_w, dimred_w, dimred_b, Wo_w, Wo_b, Wg_w, Wg_b):
    """Returns in_maps: one dict per core."""
    x = np.asarray(x, np.float32)
    conv_w = np.asarray(conv_w, np.float32)
    dimred_w = np.asarray(dimred_w, np.float32)
    dimred_b = np.asarray(dimred_b, np.float32)
    Wo_w = np.asarray(Wo_w, np.float32)
    Wo_b = np.asarray(Wo_b, np.float32)
    Wg_w = np.asarray(Wg_w, np.float32)
    Wg_b = np.asarray(Wg_b, np.float32)

    # x_t[nh, nl, hw, b] = x[b, nh*128+nl, hw]  (partition-major contiguous)
    xt = x.transpose(1, 2, 3, 0).reshape(N, HW, B)          # [n, hw, b]
    xt = _bf16(xt.reshape(2, 128, HWP, B))

    # conv_w^T [2, 128, K]
    cwt = _bf16(conv_w.T.reshape(2, 128, K))

    shared = {
        "x_t": xt,
        "cw_t": cwt,
        "ones8": _bf16(np.ones((8, 8), np.float32)),
        "hsel": np.repeat(np.eye(K, dtype=np.float32), H, axis=0),
        "ones14": np.ones((14, 1), np.float32),
        "mones8": -np.ones((8, 1), np.float32),
        "onesb": np.ones((1, B), np.float32),
        "sel4": np.tile(np.eye(B, dtype=np.float32), (4, 1)),
    }

    in_maps = []
    for k in range(NCORES):
        # wd_t[nl, nh, hw, m] = dimred_w[k, m, n, hw]  (partition-outermost)
        wd = dimred_w[k].reshape(N, N, HW).transpose(1, 2, 0)   # [n, hw, m]
        wd = wd.reshape(2, 128, HWP, N).transpose(1, 0, 2, 3)   # [128, 2, hw, m]
        wo = np.zeros((CP, N), np.float32)
        wo[:C] = Wo_w[k]
        wob = np.zeros((CP,), np.float32)
        wob[:C] = Wo_b[k]
        selk = np.zeros((8, 128), np.float32)
        selk[k] = 1.0
        m = dict(shared)
        m.update({
            "wd_t": _bf16(wd),
            "wo_t": _bf16(wo.T.reshape(2, 128, CP)),
            "wob_t": wob.reshape(CP // 128, 128),
            "wg_t": np.ascontiguousarray(Wg_w[k].reshape(2, 128, 1)),
            "wgb_t": np.full((1, 1), Wg_b[k], np.float32),
            "db_t": dimred_b[k].reshape(1, N),
            "selk": _bf16(selk),
        })
        in_maps.append(m)
    return in_maps


def assemble_outputs(results):
    hyp = np.stack(
        [r["out_hyp"].reshape(CP, B)[:C].T for r in results], axis=1
    )                                                   # [B, K, C]
    conf = np.stack([r["out_conf"][0] for r in results], axis=1)[..., None]
    loss = results[0]["out_loss"][0][:, None]           # [B, 1]
    return (
        np.ascontiguousarray(hyp, np.float32),
        np.ascontiguousarray(conf, np.float32),
        np.ascontiguousarray(loss, np.float32),
    )


_GRAPH_CACHE = {}


def get_graph():
    if "nc" not in _GRAPH_CACHE:
        nc = build_graph()
        nc.finalize()
        _GRAPH_CACHE["nc"] = nc
    return _GRAPH_CACHE["nc"]


def kernel(**inputs):
    nc = get_graph()
    in_maps = build_host_inputs(**inputs)
    res = run_bass_kernel_spmd(nc, in_maps, core_ids=list(range(NCORES)))
    return assemble_outputs(res.results)


# revision 70
# speedup vs baseline: 1.2369x; 1.0264x over previous
"""Trainium2 Bass kernel for nn_Attention_9844065042780.

Sharding: expert-parallel over the K=8 independent groups, one group per
NeuronCore (8 cores).  Each core receives the full activations x (reordered
host-side), the full conv_w (to compute the shared softmax attention maps
and the shared orthogonality loss), and only its own group's
dimred/Wo/Wg weights.  Outputs are disjoint per-core slices (hyp[:,k,:],
conf[:,k]) plus the (identical on every core) loss, gathered host-side.

Per-core math (k = this core's group):
  z[k', hw, b]   = sum_n conv_w[k', n] x[b, n, hw]            (PE)
  ah[k', hw, b]  = softmax_k'(z)                              (ACT exp, PE sum, DVE recip/mul)
  y[(hw,n), b]   = ah[k, hw, b] * x[b, n, hw]                 (PE row-broadcast + DVE mul)
  dim_red[b, m]  = sum_{hw,n} y * wd[m, n, hw] + db[m]        (PE, 393 accumulating matmuls
                   packed 4-wide across the PE column groups via tile_position)
  hyp[c, b]      = sum_n Wo[c, n] dim_red[b, n] + Wo_b[c]     (PE + ACT bias)
  conf[b]        = tanh(sum_n Wg[n] dim_red[b, n] + Wg_b)     (PE + ACT)
  loss[b]        = ||A_b^T A_b||_F^2 - sum((H^T A_b)^2),  A_b = ah[:, :, :, b]  (PE/ACT/DVE)

Pipeline: 14 hw-slices (one h row each, 448 free columns).  Emission is
software-pipelined (z two slices ahead, softmax-mid one ahead, the heavy
wd-matmul stage behind) so the PE never stalls in-order on the cross-engine
softmax chain; wd streams on the sync HWDGE ring at ~2MB per slice with the
x quarters interleaved; the loss block is emitted before the last two heavy
stages so it overlaps the final wd waits.

Layouts (host-prepared, partition-major so every DMA is contiguous
per partition):
  x_t  [2, 128, 196, 32]  bf16   x_t[nh, nl, hw, b] = x[b, nh*128+nl, hw]
  wd_t [128, 2, 196, 256] bf16   wd_t[nl, nh, hw, m] = dimred_w[k, m, nh*128+nl, hw]
"""

import os
import numpy as np
import ml_dtypes
from contextlib import ExitStack

from concourse import bass, bacc, tile, mybir
from concourse.bass_utils import run_bass_kernel_spmd

F32 = mybir.dt.float32
BF16 = mybir.dt.bfloat16
AF = mybir.ActivationFunctionType
ALU = mybir.AluOpType
AX = mybir.AxisListType

B, N, H, W, K, C = 32, 256, 14, 14, 8, 1000
HW, HWP, CP = 196, 196, 1024
SW = 14                      # hw positions per slice (one h row)
NSL = HWP // SW              # 14 slices
SLW = SW * B                 # 448 free columns per slice
NCORES = 8


def build_graph():
    nc = bacc.Bacc("TRN2", target_bir_lowering=False, debug=False)

    def inp(name, shape, dtype):
        return nc.dram_tensor(name, shape, dtype, kind="ExternalInput").ap()

    def outp(name, shape, dtype):
        return nc.dram_tensor(name, shape, dtype, kind="ExternalOutput").ap()

    x_d = inp("x_t", [2, 128, HWP, B], BF16)
    wd_d = inp("wd_t", [128, 2, HWP, N], BF16)
    cw_d = inp("cw_t", [2, 128, K], BF16)
    wo_d = inp("wo_t", [2, 128, CP], BF16)
    wob_d = inp("wob_t", [CP // 128, 128], F32)
    wg_d = inp("wg_t", [2, 128, 1], F32)
    wgb_d = inp("wgb_t", [1, 1], F32)
    db_d = inp("db_t", [1, N], F32)
    ones8_d = inp("ones8", [8, 8], BF16)
    selk_d = inp("selk", [8, 128], BF16)
    hsel_d = inp("hsel", [K * H, K], F32)
    ones14_d = inp("ones14", [14, 1], F32)
    mones8_d = inp("mones8", [8, 1], F32)
    onesb_d = inp("onesb", [1, B], F32)
    sel4_d = inp("sel4", [128, B], F32)

    hyp_d = outp("out_hyp", [CP // 128, 128, B], F32)
    conf_d = outp("out_conf", [1, B], F32)
    loss_d = outp("out_loss", [1, B], F32)

    with tile.TileContext(nc) as tc, ExitStack() as ctx:
        const = ctx.enter_context(tc.tile_pool(name="const", bufs=1))
        persist = ctx.enter_context(tc.tile_pool(name="persist", bufs=1))
        dram = ctx.enter_context(tc.tile_pool(name="dram", bufs=1, space="DRAM"))
        dr_pool = ctx.enter_context(
            tc.tile_pool(name="dr_psum", bufs=1, space="PSUM")
        )

        # PE warm-up first in program order: memset is gpsimd's first
        # instruction and the matmuls open the HAM clock gate
        # (1.2 -> 2.4 GHz) while the DMA pipeline ramps.
        with ExitStack() as wctx:
            wup = wctx.enter_context(tc.tile_pool(name="wup", bufs=1))
            wupp = wctx.enter_context(
                tc.tile_pool(name="wupp", bufs=1, space="PSUM"))
            wu_sb = wup.tile([128, 512], BF16)
            nc.gpsimd.memset(wu_sb, 0.0)
            wu_ps = wupp.tile([128, 512], F32, space="PSUM")
            for _ in range(12):
                nc.tensor.matmul(wu_ps, lhsT=wu_sb[:, 0:128], rhs=wu_sb,
                                 start=True, stop=True)

        # ---- constants into SBUF ----
        # Main-loop-critical consts go FIRST on the sync HWDGE ring (tiny);
        # tail-only consts go on the gpsimd SWDGE ring so neither the sync
        # ring (wd stream) nor the scalar engine (exp/copies) is blocked.
        cw_sb = const.tile([128, 2 * K], BF16)
        nc.sync.dma_start(cw_sb.rearrange("p (t c) -> p t c", t=2),
                          cw_d.rearrange("t p c -> p t c"))
        db_sb = const.tile([1, N], F32)
        nc.sync.dma_start(db_sb, db_d)
        ones8_sb = const.tile([8, 8], BF16)
        nc.sync.dma_start(ones8_sb, ones8_d)
        selk_sb = const.tile([8, 128], BF16)
        nc.sync.dma_start(selk_sb, selk_d)
        onesb_sb = const.tile([1, B], F32)
        nc.sync.dma_start(onesb_sb, onesb_d)

        # full attention map, all K groups: [8, (hw, b)] fp32
        ah_full = persist.tile([K, HWP * B], F32)

        # x resident in SBUF: [128, (u, hw, b)].  Loaded on the sync ring in
        # quarter-DMAs interleaved with the first wd slices (emitted in
        # stage_z below), first quarter up-front so slice 0 unblocks early.
        x_all = persist.tile([128, 2 * HWP * B], BF16)
        XH = HWP * B
        QHW = 49  # hw positions per x quarter-load (49*4 = 196)

        def load_x_quarter(q):
            lo, hi = q * QHW, (q + 1) * QHW
            for u in range(2):
                nc.sync.dma_start(
                    x_all[:, u * XH + lo * B:u * XH + hi * B],
                    x_d[u, :, lo:hi, :])

        load_x_quarter(0)

        wo_sb = const.tile([128, 2 * CP], BF16)
        nc.gpsimd.dma_start(wo_sb.rearrange("p (t c) -> p t c", t=2),
                            wo_d.rearrange("t p c -> p t c"))
        wob_sb = const.tile([128, CP // 128], F32)
        nc.gpsimd.dma_start(wob_sb, wob_d.rearrange("c p -> p c"))
        wg_sb = const.tile([128, 2], F32)
        nc.gpsimd.dma_start(wg_sb.rearrange("p (t one) -> p t one", t=2),
                            wg_d.rearrange("t p one -> p t one"))
        wgb_sb = const.tile([1, 1], F32)
        nc.gpsimd.dma_start(wgb_sb, wgb_d)
        hsel_sb = const.tile([K * H, K], F32)
        nc.gpsimd.dma_start(hsel_sb, hsel_d)
        ones14_sb = const.tile([14, 1], F32)
        nc.gpsimd.dma_start(ones14_sb, ones14_d)
        mones8_sb = const.tile([8, 1], F32)
        nc.gpsimd.dma_start(mones8_sb, mones8_d)
        sel4_sb = const.tile([128, B], F32)
        nc.gpsimd.dma_start(sel4_sb, sel4_d)

        # dim_red accumulator: four 32-partition blocks (one per PE column
        # group) accumulate concurrently; merged after the main loop.
        dimred_ps = dr_pool.tile([128, N], F32, space="PSUM")
        # bias seeding matmul into block 0: dimred[b, m] = 1 * db[m]
        nc.tensor.matmul(
            dimred_ps[0:B, :], lhsT=onesb_sb, rhs=db_sb, start=True,
            stop=False, skip_group_check=True, tile_position=(0, 0),
        )

        with ExitStack() as mctx:
            wdp = mctx.enter_context(tc.tile_pool(name="wdp", bufs=6))
            ep = mctx.enter_context(tc.tile_pool(name="ep", bufs=4))
            abf = mctx.enter_context(tc.tile_pool(name="abf", bufs=3))
            rp = mctx.enter_context(tc.tile_pool(name="rp", bufs=3))
            yp = mctx.enter_context(tc.tile_pool(name="yp", bufs=4))
            zp = mctx.enter_context(tc.tile_pool(name="zp", bufs=2, space="PSUM"))
            sp = mctx.enter_context(tc.tile_pool(name="sp", bufs=1, space="PSUM"))
            abp = mctx.enter_context(tc.tile_pool(name="abp", bufs=1, space="PSUM"))
            lp = mctx.enter_context(tc.tile_pool(name="lp", bufs=1, space="PSUM"))
            lsb = mctx.enter_context(tc.tile_pool(name="lsb", bufs=1))

            def xw(s, u):
                return x_all[:, u * HWP * B + s * SW * B:
                             u * HWP * B + (s + 1) * SW * B]

            # Software-pipelined emission: PE sees z MMs of slice s+2 and
            # ssum of s+1 between dependent ops of slice s, so it never
            # stalls in-order on the cross-engine softmax chain.
            state = {}

            def stage_z(s):
                # one 1.8MB wd DMA per slice, prefetched 2 slices ahead;
                # remaining x quarters ride between the first wd slices
                wd_sb = wdp.tile([128, 2 * SW * N], BF16, tag="wd",
                                 name=f"wd_{s}")
                nc.sync.dma_start(
                    wd_sb.rearrange("p (u t m) -> p u (t m)", u=2, t=SW),
                    wd_d[:, :, s * SW:(s + 1) * SW, :])
                if 1 <= s <= 3:
                    load_x_quarter(s)
                z_ps = zp.tile([K, SLW], F32, tag="z", space="PSUM",
                               name=f"z_{s}")
                nc.tensor.matmul(z_ps, lhsT=cw_sb[:, 0:K], rhs=xw(s, 0),
                                 start=True, stop=False)
                nc.tensor.matmul(z_ps, lhsT=cw_sb[:, K:2 * K], rhs=xw(s, 1),
                                 start=False, stop=True)
                e_sb = ep.tile([K, SLW], F32, tag="e", name=f"e_{s}")
                nc.scalar.activation(e_sb, z_ps, AF.Exp)
                # bf16 exp copy: lets the softmax-sum matmul run at 1 cyc/row
                eb_sb = ep.tile([K, SLW], BF16, tag="eb", name=f"eb_{s}")
                nc.scalar.activation(eb_sb, z_ps, AF.Exp)
                state[s] = (wd_sb, e_sb, eb_sb)

            def stage_mid(s):
                wd_sb, e_sb, eb_sb = state[s]
                ssum_ps = sp.tile([K, SLW], F32, tag="ss", space="PSUM",
                                  name=f"ss_{s}")
                nc.tensor.matmul(ssum_ps, lhsT=ones8_sb, rhs=eb_sb,
                                 start=True, stop=True)
                r_sb = rp.tile([K, SLW], F32, tag="r", name=f"r_{s}")
                nc.vector.reciprocal_approx_fast(r_sb, ssum_ps)
                ahw = ah_full[:, s * SLW:(s + 1) * SLW]
                nc.vector.tensor_tensor(out=ahw, in0=e_sb, in1=r_sb,
                                        op=ALU.mult)
                # bf16 copy (on ACT) so the broadcast matmul is 1 cyc/row
                ah_bf = abf.tile([K, SLW], BF16, tag="ahbf", name=f"abf_{s}")
                nc.scalar.copy(ah_bf, ahw)
                state[s] = (wd_sb, ah_bf)

            def stage_heavy(s):
                wd_sb, ah_bf = state.pop(s)
                ahb_ps = abp.tile([128, SLW], F32, tag="ab", space="PSUM",
                                  name=f"ab_{s}")
                nc.tensor.matmul(ahb_ps, lhsT=selk_sb, rhs=ah_bf,
                                 start=True, stop=True)
                for nh in range(2):
                    y_sb = yp.tile([128, SLW], BF16, tag="y",
                                   name=f"y_{s}_{nh}")
                    nc.vector.tensor_tensor(out=y_sb, in0=xw(s, nh),
                                            in1=ahb_ps, op=ALU.mult)
                    for j in range(SW):
                        g = (s * 2 + nh) * SW + j     # global chunk ordinal
                        blk = g % 4                   # PE column group
                        nc.tensor.matmul(
                            dimred_ps[blk * B:(blk + 1) * B, :],
                            lhsT=y_sb[:, j * B:(j + 1) * B],
                            rhs=wd_sb[:, (nh * SW + j) * N:
                                      (nh * SW + j + 1) * N],
                            start=(blk > 0 and g == blk),
                            stop=(g >= 2 * NSL * SW - 4),
                            skip_group_check=True,
                            tile_position=(0, blk * B),
                        )


            def emit_loss():
                # ---- loss (overlaps the tail of the wd stream) ----
                # regather ah to [(k h), (w b)] via DRAM bounce on gpsimd
                ah_dram = dram.tile([K, H * W * B], F32, space="DRAM")
                nc.gpsimd.dma_start(ah_dram, ah_full[:, 0:H * W * B])
                A2 = lsb.tile([K * H, W * B], F32)
                nc.gpsimd.dma_start(
                    A2, ah_dram.rearrange("k (h rest) -> (k h) rest", h=H)
                )
                A2v = A2.rearrange("p (w b) -> p w b", b=B)
                G_ps = lp.tile([W, B * W], F32, tag="G", space="PSUM")
                for b in range(B):
                    ab = A2v[:, :, b:b + 1]
                    nc.tensor.matmul(G_ps[:, b * W:(b + 1) * W], lhsT=ab,
                                     rhs=ab, start=True, stop=True,
                                     skip_group_check=True)
                S_ps = lp.tile([K, W * B], F32, tag="S", space="PSUM")
                nc.tensor.matmul(S_ps, lhsT=hsel_sb, rhs=A2,
                                 start=True, stop=True)
                Gsq = lsb.tile([W, B * W], F32)
                nc.scalar.activation(Gsq, G_ps, AF.Square)
                Ssq = lsb.tile([K, W * B], F32)
                nc.scalar.activation(Ssq, S_ps, AF.Square)
                Gred = lsb.tile([W, B], F32)
                nc.vector.tensor_reduce(
                    Gred, Gsq.rearrange("p (b v) -> p b v", b=B),
                    axis=AX.X, op=ALU.add,
                )
                Sred = lsb.tile([K, B], F32)
                nc.vector.tensor_reduce(
                    Sred, Ssq.rearrange("p (w b) -> p b w", b=B),
                    axis=AX.X, op=ALU.add,
                )
                l_ps = lp.tile([1, B], F32, tag="l", space="PSUM")
                nc.tensor.matmul(l_ps, lhsT=ones14_sb, rhs=Gred,
                                 start=True, stop=False)
                nc.tensor.matmul(l_ps, lhsT=mones8_sb, rhs=Sred,
                                 start=False, stop=True)
                loss_sb = lsb.tile([1, B], F32)
                nc.vector.tensor_copy(loss_sb, l_ps)
                nc.gpsimd.dma_start(loss_d, loss_sb)

            LEAD = 2
            for s in range(NSL + LEAD):
                if s < NSL:
                    stage_z(s)
                if 1 <= s and s - 1 < NSL:
                    stage_mid(s - 1)
                if s == NSL:
                    emit_loss()
                if s >= LEAD:
                    stage_heavy(s - LEAD)

        # ---------------- tail ----------------
        with ExitStack() as tctx:
            tp = tctx.enter_context(tc.tile_pool(name="tail_sb", bufs=1))
            tpp = tctx.enter_context(
                tc.tile_pool(name="tail_ps", bufs=1, space="PSUM")
            )
            hp = tctx.enter_context(
                tc.tile_pool(name="hyp_ps", bufs=3, space="PSUM")
            )
            hs = tctx.enter_context(tc.tile_pool(name="hyp_sb", bufs=2))

            # merge + transpose fused: drT[n, b] = sum_g drg[32g+b, n]
            # = (drg^T @ sel4), two 128-col matmuls straight to [n, b].
            drg_sb = tp.tile([128, N], F32)
            nc.vector.tensor_copy(drg_sb, dimred_ps)
            drT_sb = tp.tile([128, 2 * B], F32)
            drT_bf = tp.tile([128, 2 * B], BF16)
            for nh in range(2):
                drT_ps = tpp.tile([128, B], F32, tag="drT", space="PSUM")
                nc.tensor.matmul(drT_ps, lhsT=drg_sb[:, nh * 128:(nh + 1) * 128],
                                 rhs=sel4_sb, start=True, stop=True)
                nc.vector.tensor_copy(drT_sb[:, nh * B:(nh + 1) * B], drT_ps)
                nc.scalar.copy(drT_bf[:, nh * B:(nh + 1) * B], drT_ps)

            # hyp: all 8 c-tiles in one PSUM bank, windowed bias ACTs,
            # one output DMA (avoids pool-slot serialization in the tail)
            hyp_ps = hp.tile([128, CP // 128 * B], F32, tag="hyp",
                             space="PSUM")
            hyp_all = hs.tile([128, CP // 128 * B], F32)
            for c in range(CP // 128):
                w = hyp_ps[:, c * B:(c + 1) * B]
                nc.tensor.matmul(
                    w, lhsT=wo_sb[:, c * 128:(c + 1) * 128],
                    rhs=drT_bf[:, 0:B], start=True, stop=False,
                    skip_group_check=True,
                )
                nc.tensor.matmul(
                    w, lhsT=wo_sb[:, CP + c * 128:CP + (c + 1) * 128],
                    rhs=drT_bf[:, B:2 * B], start=False, stop=True,
                    skip_group_check=True,
                )
                nc.scalar.activation(hyp_all[:, c * B:(c + 1) * B], w,
                                     AF.Identity, bias=wob_sb[:, c:c + 1])
            nc.sync.dma_start(
                hyp_d.rearrange("c p b -> p c b"),
                hyp_all.rearrange("p (c b) -> p c b", c=CP // 128))

            # conf
            conf_ps = tpp.tile([1, B], F32, tag="conf", space="PSUM")
            nc.tensor.matmul(conf_ps, lhsT=wg_sb[:, 0:1], rhs=drT_sb[:, 0:B],
                             start=True, stop=False)
            nc.tensor.matmul(conf_ps, lhsT=wg_sb[:, 1:2], rhs=drT_sb[:, B:2 * B],
                             start=False, stop=True)
            conf_sb = tp.tile([1, B], F32)
            nc.scalar.activation(conf_sb, conf_ps, AF.Tanh, bias=wgb_sb[:, 0:1])
            nc.sync.dma_start(conf_d, conf_sb)


    return nc


def _bf16(a):
    return np.ascontiguousarray(a.astype(ml_dtypes.bfloat16))


def build_host_inputs(x, conv_w, dimred_w, dimred_b, Wo_w, Wo_b, Wg_w, Wg_b):
    """Returns in_maps: one dict per core."""
    x = np.asarray(x, np.float32)
    conv_w = np.asarray(conv_w, np.float32)
    dimred_w = np.asarray(dimred_w, np.float32)
    dimred_b = np.asarray(dimred_b, np.float32)
    Wo_w = np.asarray(Wo_w, np.float32)
    Wo_b = np.asarray(Wo_b, np.float32)
    Wg_w = np.asarray(Wg_w, np.float32)
    Wg_b = np.asarray(Wg_b, np.float32)

    # x_t[nh, nl, hw, b] = x[b, nh*128+nl, hw]  (partition-major contiguous)
    xt = x.transpose(1, 2, 3, 0).reshape(N, HW, B)          # [n, hw, b]
    xt = _bf16(xt.reshape(2, 128, HWP, B))

    # conv_w^T [2, 128, K]
    cwt = _bf16(conv_w.T.reshape(2, 128, K))

    shared = {
        "x_t": xt,
        "cw_t": cwt,
        "ones8": _bf16(np.ones((8, 8), np.float32)),
        "hsel": np.repeat(np.eye(K, dtype=np.float32), H, axis=0),
        "ones14": np.ones((14, 1), np.float32),
        "mones8": -np.ones((8, 1), np.float32),
        "onesb": np.ones((1, B), np.float32),
        "sel4": np.tile(np.eye(B, dtype=np.float32), (4, 1)),
    }

    in_maps = []
    for k in range(NCORES):
        # wd_t[nl, nh, hw, m] = dimred_w[k, m, n, hw]  (partition-outermost)
        wd = dimred_w[k].reshape(N, N, HW).transpose(1, 2, 0)   # [n, hw, m]
        wd = wd.reshape(2, 128, HWP, N).transpose(1, 0, 2, 3)   # [128, 2, hw, m]
        wo = np.zeros((CP, N), np.float32)
        wo[:C] = Wo_w[k]
        wob = np.zeros((CP,), np.float32)
        wob[:C] = Wo_b[k]
        selk = np.zeros((8, 128), np.float32)
        selk[k] = 1.0
        m = dict(shared)
        m.update({
            "wd_t": _bf16(wd),
            "wo_t": _bf16(wo.T.reshape(2, 128, CP)),
            "wob_t": wob.reshape(CP // 128, 128),
            "wg_t": np.ascontiguousarray(Wg_w[k].reshape(2, 128, 1)),
            "wgb_t": np.full((1, 1), Wg_b[k], np.float32),
            "db_t": dimred_b[k].reshape(1, N),
            "selk": _bf16(selk),
        })
        in_maps.append(m)
    return in_maps


def assemble_outputs(results):
    hyp = np.stack(
        [r["out_hyp"].reshape(CP, B)[:C].T for r in results], axis=1
    )                                                   # [B, K, C]
    conf = np.stack([r["out_conf"][0] for r in results], axis=1)[..., None]
    loss = results[0]["out_loss"][0][:, None]           # [B, 1]
    return (
        np.ascontiguousarray(hyp, np.float32),
        np.ascontiguousarray(conf, np.float32),
        np.ascontiguousarray(loss, np.float32),
    )


_GRAPH_CACHE = {}


def get_graph():
    if "nc" not in _GRAPH_CACHE:
        nc = build_graph()
        nc.finalize()
        _GRAPH_CACHE["nc"] = nc
    return _GRAPH_CACHE["nc"]


def kernel(**inputs):
    nc = get_graph()
    in_maps = build_host_inputs(**inputs)
    res = run_bass_kernel_spmd(nc, in_maps, core_ids=list(range(NCORES)))
    return assemble_outputs(res.results)
